# revision 79
# baseline (speedup 1.0000x reference)
"""Causal self-attention Trainium2 Bass kernel.

Problem: B=2, N=2048, H=16 heads, Dh=64, D=1024, fp32.
  qkv = x @ W_qkv; causal softmax(q k^T / sqrt(Dh)) @ v.

Sharding (8 cores): data-parallel on B (2) x tensor-parallel on head groups (4).
Core c handles batch b = c // 4 and heads hg*4 .. hg*4+3 where hg = c % 4.

Host-side staging (make_in_maps) pre-arranges every input into the exact SBUF
image its tile wants, so each tensor loads with ONE multi-dim DMA descriptor
(the HW DGE fans a single descriptor across all 16 DMA engines; per-descriptor
issue is ~650ns on the sequencers, and the framework's DMA-semaphore pool is
only ~10 deep, so descriptor COUNT is precious):
  xs  [2*128, 4096]  f32 chunk-major x image (i-chunks 0-1 of 512 tokens)
  x8  [2*128, 4096]  fp8e4 image of chunks 2-3
  wqs/wks [128,2048] f32 pair-major; wvs [128,2048] f32 t-major
  wq8s/wk8s/wv8s     fp8e4 copies of the same images
  outT [256, 2048]   row h*64+d, col i = out[b, i, hg*256 + h*64 + d]
Input DMAs issue from BOTH HWDGE sequencers (sync + scalar) in strict
first-need order per FIFO; xt0 is split into i-quarters so the first QK
half-pieces (256 cols) can start ~4us earlier.

Precision plan (tolerance 2e-2; measured rel err ~9e-3): rows attending few
positions (chunk 0, and chunk 1 partially) are error-critical because softmax
noise doesn't average out there; long rows are robust.
  chunk 0:    f32r QKV, bf16 S, bf16 AV  (exact path)
  chunk 1:    f32r QKV, bf16 S, fp8e4 DoubleRow AV
  chunks 2-3: fp8e4 DoubleRow QKV (x8@W8), bf16 S, fp8e4 DoubleRow AV
DoubleRow packs 2 K-tiles per pass (2x rate) — it only pays on contraction-
deep matmuls (QKV K=1024, AV K=512), NOT on S (K=64/head, column-bound).
fp8 exp tops out at ~240-448, so the fp8-path EXP carries bias=-3 (softmax is
shift-invariant; numerator and denominator both scale by e^-3).

Device algorithm per core:
  qT/kT  [dh, i] tiles via matmul(lhsT=W-slice, rhs=xT), stored bf16
  v      [i, dh] tiles stored as v-hat = [v | ones64] (fp8, + bf16 copy of
         chunk 0): the ones rows replicate the softmax denominator into the
         AV output at zero PE cost (cycles only depend on N columns)
  S^T    [j, i] via row-packed bf16 matmul pairs (K=64/head, tile_position)
  expS^T via ACT Exp (scale=1/8, fp8-path bias=-3) straight from PSUM;
         chunks>=1 write fp8 pair-tiles [128, 2 jt, 2 l, 512] for DoubleRow
  causal mask on diagonal j-tiles via gpsimd affine_select (fill 0; full-width
         on fp8 tiles so never-written leading cols are zeroed for DR streams)
  AV     out^T accumulated in PSUM; fp8 path contracts 2 j-tiles per pass
  fin    per (l, col-group): DVE copy of the denominator rows, fast
         reciprocal, multiply, out-DMA (alternating sync/scalar engines)

Scheduling: a static greedy scheduler interleaves the S/EXP, QKV, and AV
streams against estimated PE/ACT/DMA clocks; emission order is the only knob
(real sync is inserted by the Tile framework).  Constraints encoded:
  - ps_s has 2 slots, so ACT runs at most ~2 EXPs ahead; feed it S units
    regularly and never >2 in a row
  - AV pairs chain per psum pool; fins are emitted eagerly (they cost no PE
    time and un-gate the pool chains)
  - front QKV pieces gate on FULL DMA arrival; dependency-free dummy matmuls
    fill predicted PE waits so the HAM clock gate never drops to 1.2 GHz
Engine budget (measured): PE matmul ~185us sum incl. ramp, ACT EXP ~80us,
DVE ~60us; HW exec ~139us vs the 157us all-f32r/bf16 baseline.
"""

import numpy as np

import concourse.mybir as mybir
import concourse.tile as tile
from concourse import bacc
from concourse.bass_utils import run_bass_kernel_spmd

F32 = mybir.dt.float32
F32R = mybir.dt.float32r
BF16 = mybir.dt.bfloat16
FP8 = mybir.dt.float8e4
DR = mybir.MatmulPerfMode.DoubleRow

B = 2
N = 2048
D = 1024
H_PER_CORE = 4
DH = 64
NCHUNK = 4          # i-chunks of 512
CH = 512
DT = 8              # d-tiles of 128
NT = 16             # token tiles of 128
SCALE = 1.0 / 8.0   # 1/sqrt(64)
E_BUFS = 40

_CACHED_NC = None


def build_nc(debug_sched=False, skip_compile=False):
    nc = bacc.Bacc("TRN2", target_bir_lowering=False, debug=False)
    # x chunks 0-1 + weights in f32; chunks 2-3 and a weight copy in fp8e4
    # (those rows average over 1024+ positions, so fp8 QKV noise washes out)
    xs = nc.dram_tensor("xs", [2 * 128, DT * CH], BF16, kind="ExternalInput").ap()
    x8 = nc.dram_tensor("x8", [2 * 128, DT * CH], FP8, kind="ExternalInput").ap()
    wqs = nc.dram_tensor("wqs", [128, 2 * DT * 128], BF16, kind="ExternalInput").ap()
    wks = nc.dram_tensor("wks", [128, 2 * DT * 128], BF16, kind="ExternalInput").ap()
    wvs = nc.dram_tensor("wvs", [128, DT * 256], BF16, kind="ExternalInput").ap()
    wq8s = nc.dram_tensor("wq8s", [128, 2 * DT * 128], FP8, kind="ExternalInput").ap()
    wk8s = nc.dram_tensor("wk8s", [128, 2 * DT * 128], FP8, kind="ExternalInput").ap()
    wv8s = nc.dram_tensor("wv8s", [128, DT * 256], FP8, kind="ExternalInput").ap()
    outT = nc.dram_tensor("outT", [H_PER_CORE * DH, N], F32, kind="ExternalOutput").ap()

    with tile.TileContext(nc) as tc:
        with (
            tc.tile_pool(name="sb_w", bufs=1) as sb_w,
            tc.tile_pool(name="sb_x", bufs=2) as sb_x,
            tc.tile_pool(name="sb_qk", bufs=1) as sb_qk,
            tc.tile_pool(name="sb_v", bufs=1) as sb_v,
            tc.tile_pool(name="sb_e", bufs=8) as sb_e,
            tc.tile_pool(name="sb_e8", bufs=30) as sb_e8,
            tc.tile_pool(name="sb_n", bufs=2) as sb_n,
            tc.tile_pool(name="ps_av", bufs=2, space="PSUM") as ps_av,
            tc.tile_pool(name="ps_qkv", bufs=2, space="PSUM") as ps_qkv,
            tc.tile_pool(name="ps_s", bufs=2, space="PSUM") as ps_s,
        ):
            wq_sb = sb_w.tile([128, 2 * DT * 128], BF16)
            wk_sb = sb_w.tile([128, 2 * DT * 128], BF16)
            wv_sb = sb_w.tile([128, DT * 256], BF16)
            wq8_sb = sb_w.tile([128, 2 * DT * 128], FP8)
            wk8_sb = sb_w.tile([128, 2 * DT * 128], FP8)
            wv8_sb = sb_w.tile([128, DT * 256], FP8)
            xtc8_tiles = {
                2: sb_w.tile([128, DT * CH], FP8, name="xtc8_2"),
                3: sb_w.tile([128, DT * CH], FP8, name="xtc8_3"),
            }

            xtc_tiles = {}

            # --- prologue DMAs: BOTH HWDGE groups (sync + scalar) are loaded
            # with the full input stream in strict first-need order; each
            # group's FIFO then delivers in priority order, so later tensors
            # can't steal bandwidth from the critical first pieces.  Tensors
            # are split between (or across) the two streams to balance them.
            xtc0 = sb_x.tile([128, DT * CH], BF16, tag="xtc", name="xtc0")
            xtc_tiles[0] = xtc0
            # xt0 split by i-halves (NOT by t): each half is a full-K image of
            # 256 i-columns, so the first QK half-pieces can start on half A.
            xtc0_v = xtc0[:, :].rearrange("p (t i) -> p t i", i=CH)
            xs0_v = xs[0:128, :].rearrange("p (t i) -> p t i", i=CH)
            # 12 descriptors total: the framework's DMA-semaphore pool is ~10
            # deep, so more would serialize issue on semaphore recycling.
            # First data for the q00a half-piece (xt0 i 0:256 + wq half 0)
            # leads both FIFOs.
            nc.sync.dma_start(xtc0_v[:, :, 0:128], xs0_v[:, :, 0:128])
            nc.scalar.dma_start(xtc0_v[:, :, 128:256], xs0_v[:, :, 128:256])
            nc.sync.dma_start(wq_sb[:, 0:1024], wqs[:, 0:1024])
            nc.scalar.dma_start(wk_sb[:, 0:1024], wks[:, 0:1024])
            nc.sync.dma_start(xtc0_v[:, :, 256:384], xs0_v[:, :, 256:384])
            nc.scalar.dma_start(xtc0_v[:, :, 384:CH], xs0_v[:, :, 384:CH])
            nc.sync.dma_start(wq_sb[:, 1024:2048], wqs[:, 1024:2048])
            nc.scalar.dma_start(wk_sb[:, 1024:2048], wks[:, 1024:2048])
            nc.scalar.dma_start(wv_sb[:, :], wvs[:, :])
            xtc1 = sb_x.tile([128, DT * CH], BF16, tag="xtc", name="xtc1")
            xtc_tiles[1] = xtc1
            nc.sync.dma_start(xtc1[:, :], xs[128:256, :])
            nc.scalar.dma_start(wq8_sb[:, :], wq8s[:, :])
            nc.scalar.dma_start(wk8_sb[:, :], wk8s[:, :])
            nc.sync.dma_start(xtc8_tiles[2][:, :], x8[0:128, :])
            nc.sync.dma_start(xtc8_tiles[3][:, :], x8[128:256, :])
            nc.scalar.dma_start(wv8_sb[:, :], wv8s[:, :])

            # warm-up: dependency-free matmuls on zeroed SBUF lift the HAM
            # clock gate (needs ~3.4us of PE activity); memset straight into
            # F32R tiles (gpsimd, earliest-starting engine) - no cast chain.
            wz = sb_v.tile([128, 1], F32)
            nc.gpsimd.memset(wz[:], 0.0)
            ebias_sb = sb_v.tile([128, 1], F32)
            nc.gpsimd.memset(ebias_sb[:], -3.0)
            xz = sb_v.tile([128, CH], F32)
            nc.gpsimd.memset(xz[:], 0.0)
            warm_ps = ps_qkv.tile([128, CH], F32, tag="ps_qkv")
            for _ in range(14):
                nc.tensor.matmul(warm_ps[0:1, :], wz[:].bitcast(F32R), xz[:].bitcast(F32R),
                                 start=True, stop=True, skip_group_check=True)

            # persistent activations (bf16 q/k: same PE rate as f32r, half the
            # SBUF; S is column-throughput-bound so fp8 wouldn't help it)
            qt_sb = sb_qk.tile([128, 2 * N], BF16)   # [pair][chunk]
            kt_sb = sb_qk.tile([128, 2 * N], BF16)
            # v-hat per (it, head): fp8 for chunks>=1 (DoubleRow AV), bf16
            # copy of chunk-0's tiles for the exact c0 path
            vh8_sb = sb_v.tile([128, NT * H_PER_CORE * 128], FP8)
            vh8_v = vh8_sb.rearrange("p (j h w) -> p j h w", h=H_PER_CORE, w=128)
            vh16_sb = sb_v.tile([128, 4 * H_PER_CORE * 128], BF16)
            # ones columns of v-hat (denominator trick): one strided memset
            nc.vector.memset(vh8_v[:, :, :, 64:128], 1.0)
            vh16_blocks = vh16_sb.rearrange("p (b w) -> p b w", w=128)
            nc.vector.memset(vh16_blocks[:, :, 64:128], 1.0)

            # ---------------- emission thunks ----------------

            def qk_piece(c, p, which, lo=0, hi=CH):
                w = hi - lo
                dst = qt_sb if which == "q" else kt_sb
                pres = ps_qkv.tile([128, CH], F32, tag="ps_qkv")
                if c >= 2:
                    # fp8 DoubleRow: t-pairs contract K=256 per pass (2x rate)
                    xt8 = xtc8_tiles[c].rearrange("r (t i) -> r t i", i=CH)
                    w8 = wq8_sb if which == "q" else wk8_sb
                    for tp in range(4):
                        lhsT = w8[:, p * 1024 + tp * 256: p * 1024 + (tp + 1) * 256
                                  ].rearrange("r (t c2) -> r t c2", t=2)
                        nc.tensor.matmul(
                            pres[:, 0:w], lhsT, xt8[:, 2 * tp:2 * tp + 2, lo:hi],
                            start=(tp == 0), stop=(tp == 3), perf_mode=DR)
                else:
                    xtc = xtc_tiles[c]
                    w_sb = wq_sb if which == "q" else wk_sb
                    for t in range(DT):
                        nc.tensor.matmul(
                            pres[:, 0:w], w_sb[:, p * 1024 + t * 128: p * 1024 + (t + 1) * 128],
                            xtc[:, t * CH + lo: t * CH + hi],
                            start=(t == 0), stop=(t == DT - 1))
                nc.vector.tensor_copy(
                    dst[:, p * N + c * CH + lo: p * N + c * CH + hi], pres[:, 0:w])

            def v_piece(c, il):
                it = 4 * c + il
                v_ps = ps_qkv.tile([128, 256], F32, tag="ps_qkv")
                if c >= 2:
                    xt8 = xtc8_tiles[c].rearrange("r (t i) -> r t i", i=CH)
                    for tp in range(4):
                        rhs = wv8_sb[:, tp * 512:(tp + 1) * 512
                                     ].rearrange("r (t c2) -> r t c2", t=2)
                        nc.tensor.matmul(
                            v_ps[:], xt8[:, 2 * tp:2 * tp + 2, il * 128:(il + 1) * 128],
                            rhs,
                            start=(tp == 0), stop=(tp == 3), perf_mode=DR)
                else:
                    xtc = xtc_tiles[c]
                    for t in range(DT):
                        nc.tensor.matmul(
                            v_ps[:], xtc[:, t * CH + il * 128: t * CH + (il + 1) * 128],
                            wv_sb[:, t * 256:(t + 1) * 256],
                            start=(t == 0), stop=(t == DT - 1))
                # one strided cast: 4 heads' v blocks (cols 0:64 of each 128-block)
                nc.vector.tensor_copy(
                    vh8_v[:, it, :, 0:64],
                    v_ps.rearrange("p (h w) -> p h w", h=4)[:])
                if c == 0:   # exact bf16 copy for the chunk-0 AV path
                    dst = vh16_sb[:, it * 512:(it + 1) * 512]
                    nc.vector.tensor_copy(
                        dst.rearrange("p (h w) -> p h w", h=4)[:, :, 0:64],
                        v_ps.rearrange("p (h w) -> p h w", h=4)[:])

            e_tiles = {}     # c==0: bf16 [128, 1024] per (c,p,jt)
            e8_pairs = {}    # c>=1: fp8 [128, 2048] per (c,p,jt//2), layout [jtslot][l][i]

            def s_exp_jt(c, p, jt):
                d = jt - 4 * c           # >= 0 on diagonal tiles
                off = 128 * d if d > 0 else 0
                s_ps = ps_s.tile([128, 1024], F32, tag="ps_s",
                                 name=f"s_c{c}_p{p}_j{jt}")
                for l in range(2):
                    nc.tensor.matmul(
                        s_ps[:, l * CH + off:(l + 1) * CH],
                        kt_sb[l * 64:(l + 1) * 64, p * N + jt * 128: p * N + (jt + 1) * 128],
                        qt_sb[l * 64:(l + 1) * 64, p * N + c * CH + off: p * N + (c + 1) * CH],
                        start=True, stop=True,
                        tile_position=(l * 64, 0))
                if c == 0:
                    e_t = sb_e.tile([128, 1024], BF16, tag="e",
                                    name=f"e_c{c}_p{p}_j{jt}")
                    e_view = e_t[:, :]
                    e_tiles[(c, p, jt)] = e_t
                else:
                    if jt % 2 == 0:
                        e8_pairs[(c, p, jt // 2)] = sb_e8.tile(
                            [128, 2048], FP8, tag="e8", name=f"e8_c{c}_p{p}_j{jt}")
                    e_view = e8_pairs[(c, p, jt // 2)][:, (jt % 2) * 1024:(jt % 2) * 1024 + 1024]
                # fp8 e4m3 tops out at 448 < e^{max S}: bias the fp8-path exp
                # by -3 (softmax is shift-invariant; num and den both carry
                # e^-3, cancelling in the ratio)
                ebias = 0.0 if c == 0 else ebias_sb[:, 0:1]
                if off:
                    # one ACT instruction over both l-halves, skipping the
                    # fully-masked leading columns of each half
                    src = s_ps.rearrange("p (l w) -> p l w", l=2)[:, :, off:]
                    dst = e_view.rearrange("p (l w) -> p l w", l=2)[:, :, off:]
                    nc.scalar.activation(dst, src,
                                         mybir.ActivationFunctionType.Exp,
                                         scale=SCALE, bias=ebias)
                else:
                    nc.scalar.activation(e_view, s_ps[:],
                                         mybir.ActivationFunctionType.Exp,
                                         scale=SCALE, bias=ebias)
                if d >= 0:  # diagonal tile: zero where j > i
                    # c>=1 (fp8 DR) streams the FULL width, so the leading
                    # [0:off) cols must be zero-filled too: run the select
                    # full-width with base=-off (it reads-then-discards the
                    # never-written cols, writing fill there).
                    wsel = CH if c != 0 else CH - off
                    o2 = 0 if c != 0 else off
                    for l in range(2):
                        nc.gpsimd.affine_select(
                            out=e_view[:, l * CH + o2:(l + 1) * CH],
                            in_=e_view[:, l * CH + o2:(l + 1) * CH],
                            compare_op=mybir.AluOpType.is_ge,
                            fill=0.0,
                            base=-off + o2,
                            channel_multiplier=-1,
                            pattern=[[1, wsel]])

            av_tiles = {}

            def setup_pair(c, p):
                # late pairs borrow by-then-idle psum pools so multiple pairs
                # accumulate concurrently at the endgame; (3,1) gets the endgame
                pool, tag = ((ps_qkv, "ps_qkv") if (c, p) in QKV_POOL_PAIRS
                             else (ps_av, "ps_av"))
                for l in range(2):
                    av_tiles[(c, p, l)] = pool.tile(
                        [128, CH], F32, tag=tag, name=f"av_c{c}_p{p}_l{l}")

            def av_half(c, p, jt0, l):
                njt = 4 * (c + 1)
                h = p * 2 + l
                if c == 0:   # exact bf16 path
                    for jt in range(jt0, jt0 + 4):
                        dd = jt - 4 * c
                        off = 128 * dd if dd > 0 else 0
                        e_t = e_tiles[(c, p, jt)]
                        nc.tensor.matmul(
                            av_tiles[(c, p, l)][:, off:],
                            vh16_sb[:, (jt * H_PER_CORE + h) * 128: (jt * H_PER_CORE + h + 1) * 128],
                            e_t[:, l * CH + off:(l + 1) * CH],
                            start=(jt == 0),
                            stop=(jt == njt - 1),
                            skip_group_check=True)
                    return
                # fp8 DoubleRow: one matmul per jt PAIR (K = 2x128), full
                # width (masked e cols are zero-filled)
                for jp in (jt0 // 2, jt0 // 2 + 1):
                    e8 = e8_pairs[(c, p, jp)]
                    rhs = e8.rearrange("p (j l i) -> p j l i", j=2, l=2)[:, :, l, :]
                    nc.tensor.matmul(
                        av_tiles[(c, p, l)][:, :],
                        vh8_v[:, 2 * jp:2 * jp + 2, h, :],
                        rhs,
                        start=(jp == 0),
                        stop=(jp == njt // 2 - 1),
                        perf_mode=DR,
                        skip_group_check=True)

            fin_left = {}

            def fin_part(c, p, l, g, w, engine):
                # NOTE: the partition shift (rows 64:128 -> 0:64) must go through
                # tensor_copy; custom-DVE ops don't partition-shift on real HW.
                h = p * 2 + l
                av_t = av_tiles[(c, p, l)]
                s0 = g * w
                sl = slice(s0, s0 + w)
                sums_sb = sb_n.tile([64, CH], F32, tag="sums")
                nc.vector.tensor_copy(sums_sb[:, 0:w], av_t[64:128, sl])
                rc = sb_n.tile([64, CH], F32, tag="rc")
                nc.vector.reciprocal_approx_fast(rc[:, 0:w], sums_sb[:, 0:w])
                out_sb = sb_n.tile([64, CH], F32, tag="out")
                nc.vector.tensor_mul(out_sb[:, 0:w], av_t[0:64, sl], rc[:, 0:w])
                engine.dma_start(
                    outT[h * 64:(h + 1) * 64, c * CH + s0: c * CH + s0 + w],
                    out_sb[:, 0:w])
                fin_left[(c, p)] -= 1
                if fin_left[(c, p)] == 0:
                    for l2 in range(2):
                        av_tiles.pop((c, p, l2))

            # ---------------- static greedy scheduler ----------------
            # Emission order is the only knob: real sync is inserted by the
            # Tile framework.  Estimated clocks in us.
            S_COST = {0: 0.48, 1: 0.38, 2: 0.28, 3: 0.18}
            QK_COST = 1.75
            V_COST = 0.9
            AV_COST = 0.88
            EXP_COST = {0: 1.15, 1: 0.94, 2: 0.72, 3: 0.51}
            AS_LAT = 0.62
            ACT_MIN_BACKLOG = 2.4   # keep >= ~2 EXPs queued on ACT
            AV_MARGIN = 0.9

            # DMA-completion-based readiness estimates (single-descriptor DMAs,
            # ~0.30 GB/us aggregate from ~7.6us).  wv and xtc2 are kicked from
            # inside the EXP stream (see emit_s); xtc3 from sync after chunk 0
            # drains.
            qkv_queue = [
                ("q", (0, 0, 0, 256), 13.0), ("k", (0, 0, 0, 256), 13.4),
                ("q", (0, 0, 256, 512), 15.8), ("k", (0, 0, 256, 512), 16.2),
                ("q", (0, 1, 0, 256), 16.6), ("k", (0, 1, 0, 256), 17.0),
                ("q", (0, 1, 256, 512), 17.4), ("k", (0, 1, 256, 512), 17.8),
                ("v", (0, 0), 23.0), ("v", (0, 1), 23.2),
                ("v", (0, 2), 23.4), ("v", (0, 3), 23.6),
                ("q", (1, 0), 28.0), ("k", (1, 0), 28.5),
                ("q", (1, 1), 29.0), ("k", (1, 1), 29.5),
                ("v", (1, 0), 29.9), ("v", (1, 1), 30.1),
                ("q", (2, 0), 33.0), ("k", (2, 0), 33.3),
                ("q", (2, 1), 33.6), ("k", (2, 1), 33.9),
                ("v", (1, 2), 34.2), ("v", (1, 3), 34.4),
                ("v", (2, 0), 34.7), ("v", (2, 1), 34.9),
                ("q", (3, 0), 35.6), ("k", (3, 0), 35.9),
                ("v", (2, 2), 36.1), ("v", (2, 3), 36.3),
                ("q", (3, 1), 36.6), ("k", (3, 1), 36.9),
                ("v", (3, 0), 37.2), ("v", (3, 1), 37.4),
                ("v", (3, 2), 37.6), ("v", (3, 3), 37.8),
            ]

            s_queue = []
            for c in range(NCHUNK):
                for p in range(2):
                    for jt in range(4 * (c + 1)):
                        s_queue.append((c, p, jt))

            # Per-pair AV queues.  Pairs on different psum pools can interleave;
            # same-pool pairs are chained (setup allocates the pool slots, so it
            # must follow the predecessor pair's last fin).  ps_qkv-hosted pairs
            # also wait for the last QKV allocation.
            PAIR_ORDER = [(0, 0), (0, 1), (1, 0), (1, 1),
                          (2, 0), (2, 1), (3, 0), (3, 1)]
            QKV_POOL_PAIRS = ((2, 1), (3, 1))
            pair_pred = {}
            last_on = {"ps_av": None, "ps_qkv": None}
            for pr in PAIR_ORDER:
                pool = "ps_qkv" if pr in QKV_POOL_PAIRS else "ps_av"
                pair_pred[pr] = last_on[pool]
                last_on[pool] = pr
            pair_items = {}
            for (c, p) in PAIR_ORDER:
                items = [("setup", None)]
                njt = 4 * (c + 1)
                nfin = 2 if c >= 2 else 1   # fin reads split 256-wide on late chunks
                for jt0 in range(0, njt, 4):
                    last = jt0 == njt - 4
                    for l in range(2):
                        items.append(("av", (jt0, l)))
                        if last:
                            for g in range(nfin):
                                items.append(("fin", (l, g, CH // nfin)))
                pair_items[(c, p)] = items
                fin_left[(c, p)] = 2 * nfin
            pair_idx = {pr: 0 for pr in PAIR_ORDER}
            fins_emitted = set()

            qk_done = {}        # (c, p, 'q'/'k') -> est emission T_pe
            v_done = set()      # (c, il) emitted
            act_done = {}       # s unit -> est ACT completion time
            T_pe = 13.8         # est PE clock after warmup
            T_act = 0.0         # ACT busy-until
            e_inflight = 0
            qi = si = 0
            sched_log = []

            def emit_dummies(gap):
                """Fill an estimated PE stall with dependency-free matmuls so
                the HAM clock gate never sees an idle window.  Cold dummies
                (front, pre-HAM-lift) run ~0.43us, warm ~0.21us."""
                nonlocal T_pe
                cost = 0.43 if T_pe < 20.0 else 0.21
                n = min(26, int(gap * 1.02 / cost))
                if n <= 0:
                    return
                dtile = ps_s.tile([128, 1024], F32, tag="ps_s", name="dummy")
                for _ in range(n):
                    nc.tensor.matmul(dtile[0:1, 0:512],
                                     wz[:].bitcast(F32R), xz[:].bitcast(F32R),
                                     start=True, stop=True, skip_group_check=True)
                T_pe += n * cost
                sched_log.append((round(T_pe, 1), "dummy", n))

            def s_ready():
                c, p, jt = s_queue[si]
                if e_inflight >= E_BUFS - 2:
                    return False
                if (c, p, "q") not in qk_done or (jt // 4, p, "k") not in qk_done:
                    return False
                # PSUM->SBUF copy of qt/kt trails the piece matmuls by ~0.9us
                return T_pe >= max(qk_done[(c, p, "q")], qk_done[(jt // 4, p, "k")]) + 0.9

            def av_total_left():
                return sum(len(pair_items[pr]) - pair_idx[pr] for pr in PAIR_ORDER)

            def pair_open(pr):
                if pair_pred[pr] is not None and pair_pred[pr] not in fins_emitted:
                    return False
                if pr in QKV_POOL_PAIRS and qi < len(qkv_queue):
                    return False
                return True

            def next_av(relaxed=False):
                """Pending fins first (they cost no PE time and un-gate the
                psum chains), then the ready pair with the MOST progress —
                racing pairs to completion gets their fins (DVE normalize +
                out-DMA) flowing while later pairs still have PE work."""
                for pr in PAIR_ORDER:
                    idx = pair_idx[pr]
                    if idx < len(pair_items[pr]) and pair_items[pr][idx][0] == "fin":
                        return pr
                for pr in sorted(PAIR_ORDER, key=lambda pr: (pair_idx[pr], PAIR_ORDER.index(pr))):
                    idx = pair_idx[pr]
                    items = pair_items[pr]
                    if idx >= len(items):
                        continue
                    kind, args = items[idx]
                    if kind == "setup":
                        if not pair_open(pr):
                            continue
                        # setup is free; look through to the first av half
                        if len(items) > 1:
                            kind, args = items[idx + 1]
                        else:
                            return pr
                    if kind == "fin":
                        return pr
                    jt0, l = args
                    c, p = pr
                    if any((c, p, jt) not in act_done for jt in range(jt0, jt0 + 4)):
                        continue
                    if any((jt0 // 4, il) not in v_done for il in range(4)):
                        continue
                    if not relaxed and not (
                            act_done[(c, p, jt0 + 3)] + AS_LAT <= T_pe + AV_MARGIN):
                        continue
                    return pr
                return None

            def emit_qkv():
                nonlocal qi, T_pe
                kind, args, rt = qkv_queue[qi]
                qi += 1
                if rt > T_pe:
                    emit_dummies(rt - T_pe)
                T_pe = max(T_pe, rt)
                if kind in ("q", "k"):
                    if len(args) == 4:
                        c, p, lo, hi = args
                        qk_piece(c, p, kind, lo, hi)
                        T_pe += QK_COST * (hi - lo) / CH
                        if hi == CH:
                            qk_done[(c, p, kind)] = T_pe
                    else:
                        c, p = args
                        qk_piece(c, p, kind)
                        T_pe += QK_COST if c < 2 else QK_COST * 0.55
                        qk_done[(c, p, kind)] = T_pe
                else:
                    c, il = args
                    v_piece(c, il)
                    v_done.add((c, il))
                    T_pe += V_COST if c < 2 else V_COST * 0.55
                sched_log.append((round(T_pe, 1), kind, args))

            act_done_seq = []   # est ACT completion per emitted s unit, in order
            act_idle = [0.0]

            def emit_s():
                nonlocal si, T_pe, T_act, e_inflight
                unit = s_queue[si]
                si += 1
                c, p, jt = unit
                s_exp_jt(c, p, jt)
                d = jt - 4 * c
                # ps_s has 2 slots: this unit's matmuls wait until the EXP two
                # units ago has drained its slot
                if len(act_done_seq) >= 2:
                    T_pe = max(T_pe, act_done_seq[-2])
                T_pe += S_COST.get(d, 0.48)
                if T_act > 0 and T_pe > T_act:
                    act_idle[0] += T_pe - T_act
                T_act = max(T_act, T_pe) + EXP_COST.get(d, 1.15)
                act_done[unit] = T_act
                act_done_seq.append(T_act)
                e_inflight += 1
                sched_log.append((round(T_pe, 1), "s", unit, round(T_act, 1)))

            def emit_av(pr, relaxed=False):
                nonlocal T_pe, e_inflight
                c, p = pr
                kind, args = pair_items[pr][pair_idx[pr]]
                pair_idx[pr] += 1
                if kind == "setup":
                    setup_pair(c, p)
                    if pair_idx[pr] < len(pair_items[pr]):
                        emit_av(pr, relaxed)   # fall through to the first half
                elif kind == "fin":
                    l, g, w = args
                    # late fins alternate out-DMA engines: the sync HWDGE
                    # group processes descriptors serially, so the 2MB output
                    # stream would otherwise drain ~6us past the last matmul
                    eng = nc.scalar if (c >= 2 and g == 1) else nc.sync
                    fin_part(c, p, l, g, w, eng)
                    sched_log.append((round(T_pe, 1), "fin", (c, p, l, g)))
                else:
                    jt0, l = args
                    if relaxed:
                        t_need = act_done[(c, p, jt0 + 3)] + AS_LAT
                        if t_need > T_pe:
                            emit_dummies(t_need - T_pe)
                        T_pe = max(T_pe, t_need)
                    av_half(c, p, jt0, l)
                    if l == 1:
                        e_inflight -= 4
                    T_pe += AV_COST if c == 0 else AV_COST * 0.52
                    sched_log.append((round(T_pe, 1), "av", (c, p, jt0, l)))
                if pair_idx[pr] >= len(pair_items[pr]):
                    fins_emitted.add(pr)

            last_s_T = 0.0
            consec_s = 0
            other_toggle = 0
            while qi < len(qkv_queue) or si < len(s_queue) or av_total_left():
                # dma kicks are free: drain them whenever they reach the head
                while qi < len(qkv_queue) and qkv_queue[qi][0] in ("dma", "dmav"):
                    emit_qkv()
                backlog = T_act - T_pe
                s_ok = si < len(s_queue) and s_ready()
                qkv_ok = qi < len(qkv_queue) and T_pe >= qkv_queue[qi][2] - 0.5
                av_pr = next_av()
                # ACT holds at most ~2 runnable EXPs (2 ps_s slots), so it must
                # be re-fed at least every ~1.8us of PE work; and >2 s units in
                # a row stall the PE on EXP drain.
                s_want = backlog < ACT_MIN_BACKLOG or T_pe - last_s_T > 1.8
                if s_ok and s_want and (consec_s < 2 or not (qkv_ok or av_pr)):
                    emit_s()
                    last_s_T = T_pe
                    consec_s += 1
                    continue
                e_pressure = e_inflight >= E_BUFS - 8
                if e_pressure and av_pr is not None:
                    emit_av(av_pr)
                elif qkv_ok and av_pr is not None:
                    # both streams ready: alternate so neither the QKV stream
                    # nor the 6-pair-deep ps_av chain falls behind
                    if other_toggle % 2 == 0:
                        emit_qkv()
                    else:
                        emit_av(av_pr)
                    other_toggle += 1
                elif qkv_ok:
                    emit_qkv()
                elif av_pr is not None:
                    emit_av(av_pr)
                elif s_ok:
                    emit_s()
                    last_s_T = T_pe
                elif qi < len(qkv_queue):
                    emit_qkv()   # forced: waits on DMA est
                else:
                    pr = next_av(relaxed=True)
                    if pr is None:
                        raise RuntimeError("scheduler stuck")
                    emit_av(pr, relaxed=True)
                consec_s = 0

            if debug_sched:
                for entry in sched_log:
                    print(entry)
                print(f"est T_pe={T_pe:.1f} T_act={T_act:.1f} act_idle={act_idle[0]:.1f}")

    if not skip_compile:
        nc.compile()
    return nc


def _get_nc():
    global _CACHED_NC
    if _CACHED_NC is None:
        _CACHED_NC = build_nc()
    return _CACHED_NC


def make_in_maps(x, W_qkv):
    import ml_dtypes
    f8 = ml_dtypes.float8_e4m3
    bf = ml_dtypes.bfloat16
    x = np.ascontiguousarray(np.asarray(x, dtype=np.float32))
    W = np.ascontiguousarray(np.asarray(W_qkv, dtype=np.float32))
    in_maps = []
    for core in range(8):
        b, hg = core // 4, core % 4
        cols = slice(hg * 256, (hg + 1) * 256)
        xT = x[b].T                                    # [1024, 2048]
        xst = xT.reshape(8, 128, 4, 512).transpose(2, 1, 0, 3).reshape(512, 4096)
        Wq = W[:, 0 * D:1 * D][:, cols]
        Wk = W[:, 1 * D:2 * D][:, cols]
        Wv = W[:, 2 * D:3 * D][:, cols]
        wqs = np.ascontiguousarray(
            Wq.reshape(8, 128, 2, 128).transpose(1, 2, 0, 3).reshape(128, 2048))
        wks = np.ascontiguousarray(
            Wk.reshape(8, 128, 2, 128).transpose(1, 2, 0, 3).reshape(128, 2048))
        wvs = np.ascontiguousarray(
            Wv.reshape(8, 128, 256).transpose(1, 0, 2).reshape(128, 2048))
        in_maps.append({
            "xs": np.ascontiguousarray(xst[0:256]).astype(bf),
            "x8": np.ascontiguousarray(xst[256:512]).astype(f8),
            "wqs": wqs.astype(bf), "wks": wks.astype(bf), "wvs": wvs.astype(bf),
            "wq8s": wqs.astype(f8), "wk8s": wks.astype(f8),
            "wv8s": wvs.astype(f8),
        })
    return in_maps


def kernel(x, W_qkv, _res_hook=None):
    nc = _get_nc()
    in_maps = make_in_maps(x, W_qkv)
    res = run_bass_kernel_spmd(nc, in_maps, list(range(8)))
    if _res_hook is not None:
        _res_hook(res)
    out = np.empty((B, N, D), dtype=np.float32)
    for core in range(8):
        b, hg = core // 4, core % 4
        out[b, :, hg * 256:(hg + 1) * 256] = res.results[core]["outT"].T
    return out


# revision 83
# speedup vs baseline: 1.0065x; 1.0065x over previous
"""Causal self-attention Trainium2 Bass kernel.

Problem: B=2, N=2048, H=16 heads, Dh=64, D=1024, fp32.
  qkv = x @ W_qkv; causal softmax(q k^T / sqrt(Dh)) @ v.

Sharding (8 cores): data-parallel on B (2) x tensor-parallel on head groups (4).
Core c handles batch b = c // 4 and heads hg*4 .. hg*4+3 where hg = c % 4.

Host-side staging (make_in_maps) pre-arranges every input into the exact SBUF
image its tile wants, so each tensor loads with ONE multi-dim DMA descriptor
(the HW DGE fans a single descriptor across all 16 DMA engines; per-descriptor
issue is ~650ns on the sequencers, and the framework's DMA-semaphore pool is
only ~10 deep, so descriptor COUNT is precious):
  xs  [2*128, 4096]  f32 chunk-major x image (i-chunks 0-1 of 512 tokens)
  x8  [2*128, 4096]  fp8e4 image of chunks 2-3
  wqs/wks [128,2048] f32 pair-major; wvs [128,2048] f32 t-major
  wq8s/wk8s/wv8s     fp8e4 copies of the same images
  outT [256, 2048]   row h*64+d, col i = out[b, i, hg*256 + h*64 + d]
Input DMAs issue from BOTH HWDGE sequencers (sync + scalar) in strict
first-need order per FIFO; xt0 is split into i-quarters so the first QK
half-pieces (256 cols) can start ~4us earlier.

Precision plan (tolerance 2e-2; measured rel err ~9e-3): rows attending few
positions (chunk 0, and chunk 1 partially) are error-critical because softmax
noise doesn't average out there; long rows are robust.
  chunk 0:    f32r QKV, bf16 S, bf16 AV  (exact path)
  chunk 1:    f32r QKV, bf16 S, fp8e4 DoubleRow AV
  chunks 2-3: fp8e4 DoubleRow QKV (x8@W8), bf16 S, fp8e4 DoubleRow AV
DoubleRow packs 2 K-tiles per pass (2x rate) — it only pays on contraction-
deep matmuls (QKV K=1024, AV K=512), NOT on S (K=64/head, column-bound).
fp8 exp tops out at ~240-448, so the fp8-path EXP carries bias=-3 (softmax is
shift-invariant; numerator and denominator both scale by e^-3).

Device algorithm per core:
  qT/kT  [dh, i] tiles via matmul(lhsT=W-slice, rhs=xT), stored bf16
  v      [i, dh] tiles stored as v-hat = [v | ones64] (fp8, + bf16 copy of
         chunk 0): the ones rows replicate the softmax denominator into the
         AV output at zero PE cost (cycles only depend on N columns)
  S^T    [j, i] via row-packed bf16 matmul pairs (K=64/head, tile_position)
  expS^T via ACT Exp (scale=1/8, fp8-path bias=-3) straight from PSUM;
         chunks>=1 write fp8 pair-tiles [128, 2 jt, 2 l, 512] for DoubleRow
  causal mask on diagonal j-tiles via gpsimd affine_select (fill 0; full-width
         on fp8 tiles so never-written leading cols are zeroed for DR streams)
  AV     out^T accumulated in PSUM; fp8 path contracts 2 j-tiles per pass
  fin    per (l, col-group): DVE copy of the denominator rows, fast
         reciprocal, multiply, out-DMA (alternating sync/scalar engines)

Scheduling: a static greedy scheduler interleaves the S/EXP, QKV, and AV
streams against estimated PE/ACT/DMA clocks; emission order is the only knob
(real sync is inserted by the Tile framework).  Constraints encoded:
  - ps_s has 2 slots, so ACT runs at most ~2 EXPs ahead; feed it S units
    regularly and never >2 in a row
  - AV pairs chain per psum pool; fins are emitted eagerly (they cost no PE
    time and un-gate the pool chains)
  - front QKV pieces gate on FULL DMA arrival; dependency-free dummy matmuls
    fill predicted PE waits so the HAM clock gate never drops to 1.2 GHz
Engine budget (measured): PE matmul ~185us sum incl. ramp, ACT EXP ~80us,
DVE ~60us; HW exec ~139us vs the 157us all-f32r/bf16 baseline.
"""

import numpy as np

import concourse.mybir as mybir
import concourse.tile as tile
from concourse import bacc
from concourse.bass_utils import run_bass_kernel_spmd

F32 = mybir.dt.float32
F32R = mybir.dt.float32r
BF16 = mybir.dt.bfloat16
FP8 = mybir.dt.float8e4
DR = mybir.MatmulPerfMode.DoubleRow

B = 2
N = 2048
D = 1024
H_PER_CORE = 4
DH = 64
NCHUNK = 4          # i-chunks of 512
CH = 512
DT = 8              # d-tiles of 128
NT = 16             # token tiles of 128
SCALE = 1.0 / 8.0   # 1/sqrt(64)
E_BUFS = 40

_CACHED_NC = None


def build_nc(debug_sched=False, skip_compile=False):
    nc = bacc.Bacc("TRN2", target_bir_lowering=False, debug=False)
    # x chunks 0-1 + weights in f32; chunks 2-3 and a weight copy in fp8e4
    # (those rows average over 1024+ positions, so fp8 QKV noise washes out)
    xs = nc.dram_tensor("xs", [2 * 128, DT * CH], BF16, kind="ExternalInput").ap()
    x8 = nc.dram_tensor("x8", [2 * 128, DT * CH], FP8, kind="ExternalInput").ap()
    wqs = nc.dram_tensor("wqs", [128, 2 * DT * 128], BF16, kind="ExternalInput").ap()
    wks = nc.dram_tensor("wks", [128, 2 * DT * 128], BF16, kind="ExternalInput").ap()
    wvs = nc.dram_tensor("wvs", [128, DT * 256], BF16, kind="ExternalInput").ap()
    wq8s = nc.dram_tensor("wq8s", [128, 2 * DT * 128], FP8, kind="ExternalInput").ap()
    wk8s = nc.dram_tensor("wk8s", [128, 2 * DT * 128], FP8, kind="ExternalInput").ap()
    wv8s = nc.dram_tensor("wv8s", [128, DT * 256], FP8, kind="ExternalInput").ap()
    outT = nc.dram_tensor("outT", [H_PER_CORE * DH, N], F32, kind="ExternalOutput").ap()

    with tile.TileContext(nc) as tc:
        with (
            tc.tile_pool(name="sb_w", bufs=1) as sb_w,
            tc.tile_pool(name="sb_x", bufs=2) as sb_x,
            tc.tile_pool(name="sb_qk", bufs=1) as sb_qk,
            tc.tile_pool(name="sb_v", bufs=1) as sb_v,
            tc.tile_pool(name="sb_e", bufs=8) as sb_e,
            tc.tile_pool(name="sb_e8", bufs=30) as sb_e8,
            tc.tile_pool(name="sb_n", bufs=2) as sb_n,
            tc.tile_pool(name="ps_av", bufs=2, space="PSUM") as ps_av,
            tc.tile_pool(name="ps_qkv", bufs=2, space="PSUM") as ps_qkv,
            tc.tile_pool(name="ps_s", bufs=2, space="PSUM") as ps_s,
        ):
            wq_sb = sb_w.tile([128, 2 * DT * 128], F32R)
            wk_sb = sb_w.tile([128, 2 * DT * 128], F32R)
            wv_sb = sb_w.tile([128, DT * 256], F32R)
            # bf16 HALF-VIEWS of the f32r-sized tiles: the allocation (and
            # hence the whole SBUF layout) stays bit-identical to the f32
            # build, but only half the bytes are DMA'd and read.  (A plain
            # bf16 resize shifts every later tile and slows ALL EXPs ~+220ns
            # via SBUF conflicts — measured.)
            wq_bv = wq_sb[:, :].bitcast(BF16)[:, 0:2 * DT * 128]
            wk_bv = wk_sb[:, :].bitcast(BF16)[:, 0:2 * DT * 128]
            wv_bv = wv_sb[:, :].bitcast(BF16)[:, 0:DT * 256]
            wq8_sb = sb_w.tile([128, 2 * DT * 128], FP8)
            wk8_sb = sb_w.tile([128, 2 * DT * 128], FP8)
            wv8_sb = sb_w.tile([128, DT * 256], FP8)
            xtc8_tiles = {
                2: sb_w.tile([128, DT * CH], FP8, name="xtc8_2"),
                3: sb_w.tile([128, DT * CH], FP8, name="xtc8_3"),
            }

            xtc_tiles = {}

            # --- prologue DMAs: BOTH HWDGE groups (sync + scalar) are loaded
            # with the full input stream in strict first-need order; each
            # group's FIFO then delivers in priority order, so later tensors
            # can't steal bandwidth from the critical first pieces.  Tensors
            # are split between (or across) the two streams to balance them.
            xtc0 = sb_x.tile([128, DT * CH], F32R, tag="xtc", name="xtc0")
            xtc_tiles[0] = xtc0
            # xt0 split by i-halves (NOT by t): each half is a full-K image of
            # 256 i-columns, so the first QK half-pieces can start on half A.
            xtc0_bv = xtc0[:, :].bitcast(BF16)[:, 0:DT * CH]
            xtc0_v = xtc0_bv.rearrange("p (t i) -> p t i", i=CH)
            xs0_v = xs[0:128, :].rearrange("p (t i) -> p t i", i=CH)
            # 12 descriptors total: the framework's DMA-semaphore pool is ~10
            # deep, so more would serialize issue on semaphore recycling.
            # First data for the q00a half-piece (xt0 i 0:256 + wq half 0)
            # leads both FIFOs.
            nc.sync.dma_start(xtc0_v[:, :, 0:128], xs0_v[:, :, 0:128])
            nc.scalar.dma_start(xtc0_v[:, :, 128:256], xs0_v[:, :, 128:256])
            nc.sync.dma_start(wq_bv[:, 0:1024], wqs[:, 0:1024])
            nc.scalar.dma_start(wk_bv[:, 0:1024], wks[:, 0:1024])
            nc.sync.dma_start(xtc0_v[:, :, 256:384], xs0_v[:, :, 256:384])
            nc.scalar.dma_start(xtc0_v[:, :, 384:CH], xs0_v[:, :, 384:CH])
            nc.sync.dma_start(wq_bv[:, 1024:2048], wqs[:, 1024:2048])
            nc.scalar.dma_start(wk_bv[:, 1024:2048], wks[:, 1024:2048])
            nc.scalar.dma_start(wv_bv[:, :], wvs[:, :])
            xtc1 = sb_x.tile([128, DT * CH], F32R, tag="xtc", name="xtc1")
            xtc_tiles[1] = xtc1
            nc.sync.dma_start(xtc1[:, :].bitcast(BF16)[:, 0:DT * CH], xs[128:256, :])
            nc.scalar.dma_start(wq8_sb[:, :], wq8s[:, :])
            nc.scalar.dma_start(wk8_sb[:, :], wk8s[:, :])
            nc.sync.dma_start(xtc8_tiles[2][:, :], x8[0:128, :])
            nc.sync.dma_start(xtc8_tiles[3][:, :], x8[128:256, :])
            nc.scalar.dma_start(wv8_sb[:, :], wv8s[:, :])

            # warm-up: dependency-free matmuls on zeroed SBUF lift the HAM
            # clock gate (needs ~3.4us of PE activity); memset straight into
            # F32R tiles (gpsimd, earliest-starting engine) - no cast chain.
            wz = sb_v.tile([128, 1], F32)
            nc.gpsimd.memset(wz[:], 0.0)
            ebias_sb = sb_v.tile([128, 1], F32)
            nc.gpsimd.memset(ebias_sb[:], -3.0)
            xz = sb_v.tile([128, CH], F32)
            nc.gpsimd.memset(xz[:], 0.0)
            warm_ps = ps_qkv.tile([128, CH], F32, tag="ps_qkv")
            for _ in range(14):
                nc.tensor.matmul(warm_ps[0:1, :], wz[:].bitcast(F32R), xz[:].bitcast(F32R),
                                 start=True, stop=True, skip_group_check=True)

            # persistent activations (bf16 q/k: same PE rate as f32r, half the
            # SBUF; S is column-throughput-bound so fp8 wouldn't help it)
            qt_sb = sb_qk.tile([128, 2 * N], BF16)   # [pair][chunk]
            kt_sb = sb_qk.tile([128, 2 * N], BF16)
            # v-hat per (it, head): fp8 for chunks>=1 (DoubleRow AV), bf16
            # copy of chunk-0's tiles for the exact c0 path
            vh8_sb = sb_v.tile([128, NT * H_PER_CORE * 128], FP8)
            vh8_v = vh8_sb.rearrange("p (j h w) -> p j h w", h=H_PER_CORE, w=128)
            vh16_sb = sb_v.tile([128, 4 * H_PER_CORE * 128], BF16)
            # ones columns of v-hat (denominator trick): one strided memset
            nc.vector.memset(vh8_v[:, :, :, 64:128], 1.0)
            vh16_blocks = vh16_sb.rearrange("p (b w) -> p b w", w=128)
            nc.vector.memset(vh16_blocks[:, :, 64:128], 1.0)

            # ---------------- emission thunks ----------------

            def qk_piece(c, p, which, lo=0, hi=CH):
                w = hi - lo
                dst = qt_sb if which == "q" else kt_sb
                pres = ps_qkv.tile([128, CH], F32, tag="ps_qkv")
                if c >= 2:
                    # fp8 DoubleRow: t-pairs contract K=256 per pass (2x rate)
                    xt8 = xtc8_tiles[c].rearrange("r (t i) -> r t i", i=CH)
                    w8 = wq8_sb if which == "q" else wk8_sb
                    for tp in range(4):
                        lhsT = w8[:, p * 1024 + tp * 256: p * 1024 + (tp + 1) * 256
                                  ].rearrange("r (t c2) -> r t c2", t=2)
                        nc.tensor.matmul(
                            pres[:, 0:w], lhsT, xt8[:, 2 * tp:2 * tp + 2, lo:hi],
                            start=(tp == 0), stop=(tp == 3), perf_mode=DR)
                else:
                    xtc = xtc_tiles[c][:, :].bitcast(BF16)[:, 0:DT * CH]
                    w_bv2 = wq_bv if which == "q" else wk_bv
                    for t in range(DT):
                        nc.tensor.matmul(
                            pres[:, 0:w], w_bv2[:, p * 1024 + t * 128: p * 1024 + (t + 1) * 128],
                            xtc[:, t * CH + lo: t * CH + hi],
                            start=(t == 0), stop=(t == DT - 1))
                nc.vector.tensor_copy(
                    dst[:, p * N + c * CH + lo: p * N + c * CH + hi], pres[:, 0:w])

            def v_piece(c, il):
                it = 4 * c + il
                v_ps = ps_qkv.tile([128, 256], F32, tag="ps_qkv")
                if c >= 2:
                    xt8 = xtc8_tiles[c].rearrange("r (t i) -> r t i", i=CH)
                    for tp in range(4):
                        rhs = wv8_sb[:, tp * 512:(tp + 1) * 512
                                     ].rearrange("r (t c2) -> r t c2", t=2)
                        nc.tensor.matmul(
                            v_ps[:], xt8[:, 2 * tp:2 * tp + 2, il * 128:(il + 1) * 128],
                            rhs,
                            start=(tp == 0), stop=(tp == 3), perf_mode=DR)
                else:
                    xtc = xtc_tiles[c][:, :].bitcast(BF16)[:, 0:DT * CH]
                    for t in range(DT):
                        nc.tensor.matmul(
                            v_ps[:], xtc[:, t * CH + il * 128: t * CH + (il + 1) * 128],
                            wv_bv[:, t * 256:(t + 1) * 256],
                            start=(t == 0), stop=(t == DT - 1))
                # one strided cast: 4 heads' v blocks (cols 0:64 of each 128-block)
                nc.vector.tensor_copy(
                    vh8_v[:, it, :, 0:64],
                    v_ps.rearrange("p (h w) -> p h w", h=4)[:])
                if c == 0:   # exact bf16 copy for the chunk-0 AV path
                    dst = vh16_sb[:, it * 512:(it + 1) * 512]
                    nc.vector.tensor_copy(
                        dst.rearrange("p (h w) -> p h w", h=4)[:, :, 0:64],
                        v_ps.rearrange("p (h w) -> p h w", h=4)[:])

            e_tiles = {}     # c==0: bf16 [128, 1024] per (c,p,jt)
            e8_pairs = {}    # c>=1: fp8 [128, 2048] per (c,p,jt//2), layout [jtslot][l][i]

            def s_exp_jt(c, p, jt):
                d = jt - 4 * c           # >= 0 on diagonal tiles
                off = 128 * d if d > 0 else 0
                s_ps = ps_s.tile([128, 1024], F32, tag="ps_s",
                                 name=f"s_c{c}_p{p}_j{jt}")
                for l in range(2):
                    nc.tensor.matmul(
                        s_ps[:, l * CH + off:(l + 1) * CH],
                        kt_sb[l * 64:(l + 1) * 64, p * N + jt * 128: p * N + (jt + 1) * 128],
                        qt_sb[l * 64:(l + 1) * 64, p * N + c * CH + off: p * N + (c + 1) * CH],
                        start=True, stop=True,
                        tile_position=(l * 64, 0))
                if c == 0:
                    e_t = sb_e.tile([128, 1024], BF16, tag="e",
                                    name=f"e_c{c}_p{p}_j{jt}")
                    e_view = e_t[:, :]
                    e_tiles[(c, p, jt)] = e_t
                else:
                    if jt % 2 == 0:
                        e8_pairs[(c, p, jt // 2)] = sb_e8.tile(
                            [128, 2048], FP8, tag="e8", name=f"e8_c{c}_p{p}_j{jt}")
                    e_view = e8_pairs[(c, p, jt // 2)][:, (jt % 2) * 1024:(jt % 2) * 1024 + 1024]
                # fp8 e4m3 tops out at 448 < e^{max S}: bias the fp8-path exp
                # by -3 (softmax is shift-invariant; num and den both carry
                # e^-3, cancelling in the ratio)
                ebias = 0.0 if c == 0 else ebias_sb[:, 0:1]
                if off:
                    # one ACT instruction over both l-halves, skipping the
                    # fully-masked leading columns of each half
                    src = s_ps.rearrange("p (l w) -> p l w", l=2)[:, :, off:]
                    dst = e_view.rearrange("p (l w) -> p l w", l=2)[:, :, off:]
                    nc.scalar.activation(dst, src,
                                         mybir.ActivationFunctionType.Exp,
                                         scale=SCALE, bias=ebias)
                else:
                    nc.scalar.activation(e_view, s_ps[:],
                                         mybir.ActivationFunctionType.Exp,
                                         scale=SCALE, bias=ebias)
                if d >= 0:  # diagonal tile: zero where j > i
                    # c>=1 (fp8 DR) streams the FULL width, so the leading
                    # [0:off) cols must be zero-filled too: run the select
                    # full-width with base=-off (it reads-then-discards the
                    # never-written cols, writing fill there).
                    wsel = CH if c != 0 else CH - off
                    o2 = 0 if c != 0 else off
                    for l in range(2):
                        nc.gpsimd.affine_select(
                            out=e_view[:, l * CH + o2:(l + 1) * CH],
                            in_=e_view[:, l * CH + o2:(l + 1) * CH],
                            compare_op=mybir.AluOpType.is_ge,
                            fill=0.0,
                            base=-off + o2,
                            channel_multiplier=-1,
                            pattern=[[1, wsel]])

            av_tiles = {}

            def setup_pair(c, p):
                # late pairs borrow by-then-idle psum pools so multiple pairs
                # accumulate concurrently at the endgame; (3,1) gets the endgame
                pool, tag = ((ps_qkv, "ps_qkv") if (c, p) in QKV_POOL_PAIRS
                             else (ps_av, "ps_av"))
                for l in range(2):
                    av_tiles[(c, p, l)] = pool.tile(
                        [128, CH], F32, tag=tag, name=f"av_c{c}_p{p}_l{l}")

            def av_half(c, p, jt0, l):
                njt = 4 * (c + 1)
                h = p * 2 + l
                if c == 0:   # exact bf16 path
                    for jt in range(jt0, jt0 + 4):
                        dd = jt - 4 * c
                        off = 128 * dd if dd > 0 else 0
                        e_t = e_tiles[(c, p, jt)]
                        nc.tensor.matmul(
                            av_tiles[(c, p, l)][:, off:],
                            vh16_sb[:, (jt * H_PER_CORE + h) * 128: (jt * H_PER_CORE + h + 1) * 128],
                            e_t[:, l * CH + off:(l + 1) * CH],
                            start=(jt == 0),
                            stop=(jt == njt - 1),
                            skip_group_check=True)
                    return
                # fp8 DoubleRow: one matmul per jt PAIR (K = 2x128), full
                # width (masked e cols are zero-filled)
                for jp in (jt0 // 2, jt0 // 2 + 1):
                    e8 = e8_pairs[(c, p, jp)]
                    rhs = e8.rearrange("p (j l i) -> p j l i", j=2, l=2)[:, :, l, :]
                    nc.tensor.matmul(
                        av_tiles[(c, p, l)][:, :],
                        vh8_v[:, 2 * jp:2 * jp + 2, h, :],
                        rhs,
                        start=(jp == 0),
                        stop=(jp == njt // 2 - 1),
                        perf_mode=DR,
                        skip_group_check=True)

            fin_left = {}

            def fin_part(c, p, l, g, w, engine):
                # NOTE: the partition shift (rows 64:128 -> 0:64) must go through
                # tensor_copy; custom-DVE ops don't partition-shift on real HW.
                h = p * 2 + l
                av_t = av_tiles[(c, p, l)]
                s0 = g * w
                sl = slice(s0, s0 + w)
                sums_sb = sb_n.tile([64, CH], F32, tag="sums")
                nc.vector.tensor_copy(sums_sb[:, 0:w], av_t[64:128, sl])
                rc = sb_n.tile([64, CH], F32, tag="rc")
                nc.vector.reciprocal_approx_fast(rc[:, 0:w], sums_sb[:, 0:w])
                out_sb = sb_n.tile([64, CH], F32, tag="out")
                nc.vector.tensor_mul(out_sb[:, 0:w], av_t[0:64, sl], rc[:, 0:w])
                engine.dma_start(
                    outT[h * 64:(h + 1) * 64, c * CH + s0: c * CH + s0 + w],
                    out_sb[:, 0:w])
                fin_left[(c, p)] -= 1
                if fin_left[(c, p)] == 0:
                    for l2 in range(2):
                        av_tiles.pop((c, p, l2))

            # ---------------- static greedy scheduler ----------------
            # Emission order is the only knob: real sync is inserted by the
            # Tile framework.  Estimated clocks in us.
            S_COST = {0: 0.48, 1: 0.38, 2: 0.28, 3: 0.18}
            QK_COST = 1.75
            V_COST = 0.9
            AV_COST = 0.88
            EXP_COST = {0: 1.15, 1: 0.94, 2: 0.72, 3: 0.51}
            AS_LAT = 0.62
            ACT_MIN_BACKLOG = 2.4   # keep >= ~2 EXPs queued on ACT
            AV_MARGIN = 0.9

            # DMA-completion-based readiness estimates (single-descriptor DMAs,
            # ~0.30 GB/us aggregate from ~7.6us).  wv and xtc2 are kicked from
            # inside the EXP stream (see emit_s); xtc3 from sync after chunk 0
            # drains.
            qkv_queue = [
                ("q", (0, 0, 0, 256), 13.0), ("k", (0, 0, 0, 256), 13.4),
                ("q", (0, 0, 256, 512), 15.8), ("k", (0, 0, 256, 512), 16.2),
                ("q", (0, 1, 0, 256), 16.6), ("k", (0, 1, 0, 256), 17.0),
                ("q", (0, 1, 256, 512), 17.4), ("k", (0, 1, 256, 512), 17.8),
                ("v", (0, 0), 23.0), ("v", (0, 1), 23.2),
                ("v", (0, 2), 23.4), ("v", (0, 3), 23.6),
                ("q", (1, 0), 25.5), ("k", (1, 0), 26.0),
                ("q", (1, 1), 26.5), ("k", (1, 1), 27.0),
                ("v", (1, 0), 27.4), ("v", (1, 1), 27.6),
                ("q", (2, 0), 30.0), ("k", (2, 0), 30.3),
                ("q", (2, 1), 30.6), ("k", (2, 1), 30.9),
                ("v", (1, 2), 31.2), ("v", (1, 3), 31.4),
                ("v", (2, 0), 31.7), ("v", (2, 1), 31.9),
                ("q", (3, 0), 32.6), ("k", (3, 0), 32.9),
                ("v", (2, 2), 33.1), ("v", (2, 3), 33.3),
                ("q", (3, 1), 33.6), ("k", (3, 1), 33.9),
                ("v", (3, 0), 34.2), ("v", (3, 1), 34.4),
                ("v", (3, 2), 34.6), ("v", (3, 3), 34.8),
            ]

            s_queue = []
            for c in range(NCHUNK):
                for p in range(2):
                    for jt in range(4 * (c + 1)):
                        s_queue.append((c, p, jt))

            # Per-pair AV queues.  Pairs on different psum pools can interleave;
            # same-pool pairs are chained (setup allocates the pool slots, so it
            # must follow the predecessor pair's last fin).  ps_qkv-hosted pairs
            # also wait for the last QKV allocation.
            PAIR_ORDER = [(0, 0), (0, 1), (1, 0), (1, 1),
                          (2, 0), (2, 1), (3, 0), (3, 1)]
            QKV_POOL_PAIRS = ((2, 1), (3, 1))
            pair_pred = {}
            last_on = {"ps_av": None, "ps_qkv": None}
            for pr in PAIR_ORDER:
                pool = "ps_qkv" if pr in QKV_POOL_PAIRS else "ps_av"
                pair_pred[pr] = last_on[pool]
                last_on[pool] = pr
            pair_items = {}
            for (c, p) in PAIR_ORDER:
                items = [("setup", None)]
                njt = 4 * (c + 1)
                nfin = 2 if c >= 2 else 1   # fin reads split 256-wide on late chunks
                for jt0 in range(0, njt, 4):
                    last = jt0 == njt - 4
                    for l in range(2):
                        items.append(("av", (jt0, l)))
                        if last:
                            for g in range(nfin):
                                items.append(("fin", (l, g, CH // nfin)))
                pair_items[(c, p)] = items
                fin_left[(c, p)] = 2 * nfin
            pair_idx = {pr: 0 for pr in PAIR_ORDER}
            fins_emitted = set()

            qk_done = {}        # (c, p, 'q'/'k') -> est emission T_pe
            v_done = set()      # (c, il) emitted
            act_done = {}       # s unit -> est ACT completion time
            T_pe = 13.8         # est PE clock after warmup
            T_act = 0.0         # ACT busy-until
            e_inflight = 0
            qi = si = 0
            sched_log = []

            def emit_dummies(gap):
                """Fill an estimated PE stall with dependency-free matmuls so
                the HAM clock gate never sees an idle window.  Cold dummies
                (front, pre-HAM-lift) run ~0.43us, warm ~0.21us."""
                nonlocal T_pe
                cost = 0.43 if T_pe < 20.0 else 0.21
                n = min(26, int(gap * 1.02 / cost))
                if n <= 0:
                    return
                dtile = ps_s.tile([128, 1024], F32, tag="ps_s", name="dummy")
                for _ in range(n):
                    nc.tensor.matmul(dtile[0:1, 0:512],
                                     wz[:].bitcast(F32R), xz[:].bitcast(F32R),
                                     start=True, stop=True, skip_group_check=True)
                T_pe += n * cost
                sched_log.append((round(T_pe, 1), "dummy", n))

            def s_ready():
                c, p, jt = s_queue[si]
                if e_inflight >= E_BUFS - 2:
                    return False
                if (c, p, "q") not in qk_done or (jt // 4, p, "k") not in qk_done:
                    return False
                # PSUM->SBUF copy of qt/kt trails the piece matmuls by ~0.9us
                return T_pe >= max(qk_done[(c, p, "q")], qk_done[(jt // 4, p, "k")]) + 0.9

            def av_total_left():
                return sum(len(pair_items[pr]) - pair_idx[pr] for pr in PAIR_ORDER)

            def pair_open(pr):
                if pair_pred[pr] is not None and pair_pred[pr] not in fins_emitted:
                    return False
                if pr in QKV_POOL_PAIRS and qi < len(qkv_queue):
                    return False
                return True

            def next_av(relaxed=False):
                """Pending fins first (they cost no PE time and un-gate the
                psum chains), then the ready pair with the MOST progress —
                racing pairs to completion gets their fins (DVE normalize +
                out-DMA) flowing while later pairs still have PE work."""
                for pr in PAIR_ORDER:
                    idx = pair_idx[pr]
                    if idx < len(pair_items[pr]) and pair_items[pr][idx][0] == "fin":
                        return pr
                for pr in sorted(PAIR_ORDER, key=lambda pr: (pair_idx[pr], PAIR_ORDER.index(pr))):
                    idx = pair_idx[pr]
                    items = pair_items[pr]
                    if idx >= len(items):
                        continue
                    kind, args = items[idx]
                    if kind == "setup":
                        if not pair_open(pr):
                            continue
                        # setup is free; look through to the first av half
                        if len(items) > 1:
                            kind, args = items[idx + 1]
                        else:
                            return pr
                    if kind == "fin":
                        return pr
                    jt0, l = args
                    c, p = pr
                    if any((c, p, jt) not in act_done for jt in range(jt0, jt0 + 4)):
                        continue
                    if any((jt0 // 4, il) not in v_done for il in range(4)):
                        continue
                    if not relaxed and not (
                            act_done[(c, p, jt0 + 3)] + AS_LAT <= T_pe + AV_MARGIN):
                        continue
                    return pr
                return None

            def emit_qkv():
                nonlocal qi, T_pe
                kind, args, rt = qkv_queue[qi]
                qi += 1
                if rt > T_pe:
                    emit_dummies(rt - T_pe)
                T_pe = max(T_pe, rt)
                if kind in ("q", "k"):
                    if len(args) == 4:
                        c, p, lo, hi = args
                        qk_piece(c, p, kind, lo, hi)
                        T_pe += QK_COST * (hi - lo) / CH
                        if hi == CH:
                            qk_done[(c, p, kind)] = T_pe
                    else:
                        c, p = args
                        qk_piece(c, p, kind)
                        T_pe += QK_COST if c < 2 else QK_COST * 0.55
                        qk_done[(c, p, kind)] = T_pe
                else:
                    c, il = args
                    v_piece(c, il)
                    v_done.add((c, il))
                    T_pe += V_COST if c < 2 else V_COST * 0.55
                sched_log.append((round(T_pe, 1), kind, args))

            act_done_seq = []   # est ACT completion per emitted s unit, in order
            act_idle = [0.0]

            def emit_s():
                nonlocal si, T_pe, T_act, e_inflight
                unit = s_queue[si]
                si += 1
                c, p, jt = unit
                s_exp_jt(c, p, jt)
                d = jt - 4 * c
                # ps_s has 2 slots: this unit's matmuls wait until the EXP two
                # units ago has drained its slot
                if len(act_done_seq) >= 2:
                    T_pe = max(T_pe, act_done_seq[-2])
                T_pe += S_COST.get(d, 0.48)
                if T_act > 0 and T_pe > T_act:
                    act_idle[0] += T_pe - T_act
                T_act = max(T_act, T_pe) + EXP_COST.get(d, 1.15)
                act_done[unit] = T_act
                act_done_seq.append(T_act)
                e_inflight += 1
                sched_log.append((round(T_pe, 1), "s", unit, round(T_act, 1)))

            def emit_av(pr, relaxed=False):
                nonlocal T_pe, e_inflight
                c, p = pr
                kind, args = pair_items[pr][pair_idx[pr]]
                pair_idx[pr] += 1
                if kind == "setup":
                    setup_pair(c, p)
                    if pair_idx[pr] < len(pair_items[pr]):
                        emit_av(pr, relaxed)   # fall through to the first half
                elif kind == "fin":
                    l, g, w = args
                    # late fins alternate out-DMA engines: the sync HWDGE
                    # group processes descriptors serially, so the 2MB output
                    # stream would otherwise drain ~6us past the last matmul
                    eng = nc.scalar if (c >= 2 and g == 1) else nc.sync
                    fin_part(c, p, l, g, w, eng)
                    sched_log.append((round(T_pe, 1), "fin", (c, p, l, g)))
                else:
                    jt0, l = args
                    if relaxed:
                        t_need = act_done[(c, p, jt0 + 3)] + AS_LAT
                        if t_need > T_pe:
                            emit_dummies(t_need - T_pe)
                        T_pe = max(T_pe, t_need)
                    av_half(c, p, jt0, l)
                    if l == 1:
                        e_inflight -= 4
                    T_pe += AV_COST if c == 0 else AV_COST * 0.52
                    sched_log.append((round(T_pe, 1), "av", (c, p, jt0, l)))
                if pair_idx[pr] >= len(pair_items[pr]):
                    fins_emitted.add(pr)

            last_s_T = 0.0
            consec_s = 0
            other_toggle = 0
            while qi < len(qkv_queue) or si < len(s_queue) or av_total_left():
                # dma kicks are free: drain them whenever they reach the head
                while qi < len(qkv_queue) and qkv_queue[qi][0] in ("dma", "dmav"):
                    emit_qkv()
                backlog = T_act - T_pe
                s_ok = si < len(s_queue) and s_ready()
                qkv_ok = qi < len(qkv_queue) and T_pe >= qkv_queue[qi][2] - 0.5
                av_pr = next_av()
                # ACT holds at most ~2 runnable EXPs (2 ps_s slots), so it must
                # be re-fed at least every ~1.8us of PE work; and >2 s units in
                # a row stall the PE on EXP drain.
                s_want = backlog < ACT_MIN_BACKLOG or T_pe - last_s_T > 1.8
                if s_ok and s_want and (consec_s < 2 or not (qkv_ok or av_pr)):
                    emit_s()
                    last_s_T = T_pe
                    consec_s += 1
                    continue
                e_pressure = e_inflight >= E_BUFS - 8
                if e_pressure and av_pr is not None:
                    emit_av(av_pr)
                elif qkv_ok and av_pr is not None:
                    # both streams ready: alternate so neither the QKV stream
                    # nor the 6-pair-deep ps_av chain falls behind
                    if other_toggle % 2 == 0:
                        emit_qkv()
                    else:
                        emit_av(av_pr)
                    other_toggle += 1
                elif qkv_ok:
                    emit_qkv()
                elif av_pr is not None:
                    emit_av(av_pr)
                elif s_ok:
                    emit_s()
                    last_s_T = T_pe
                elif qi < len(qkv_queue):
                    emit_qkv()   # forced: waits on DMA est
                else:
                    pr = next_av(relaxed=True)
                    if pr is None:
                        raise RuntimeError("scheduler stuck")
                    emit_av(pr, relaxed=True)
                consec_s = 0

            if debug_sched:
                for entry in sched_log:
                    print(entry)
                print(f"est T_pe={T_pe:.1f} T_act={T_act:.1f} act_idle={act_idle[0]:.1f}")

    if not skip_compile:
        nc.compile()
    return nc


def _get_nc():
    global _CACHED_NC
    if _CACHED_NC is None:
        _CACHED_NC = build_nc()
    return _CACHED_NC


def make_in_maps(x, W_qkv):
    import ml_dtypes
    f8 = ml_dtypes.float8_e4m3
    bf = ml_dtypes.bfloat16
    x = np.ascontiguousarray(np.asarray(x, dtype=np.float32))
    W = np.ascontiguousarray(np.asarray(W_qkv, dtype=np.float32))
    in_maps = []
    for core in range(8):
        b, hg = core // 4, core % 4
        cols = slice(hg * 256, (hg + 1) * 256)
        xT = x[b].T                                    # [1024, 2048]
        xst = xT.reshape(8, 128, 4, 512).transpose(2, 1, 0, 3).reshape(512, 4096)
        Wq = W[:, 0 * D:1 * D][:, cols]
        Wk = W[:, 1 * D:2 * D][:, cols]
        Wv = W[:, 2 * D:3 * D][:, cols]
        wqs = np.ascontiguousarray(
            Wq.reshape(8, 128, 2, 128).transpose(1, 2, 0, 3).reshape(128, 2048))
        wks = np.ascontiguousarray(
            Wk.reshape(8, 128, 2, 128).transpose(1, 2, 0, 3).reshape(128, 2048))
        wvs = np.ascontiguousarray(
            Wv.reshape(8, 128, 256).transpose(1, 0, 2).reshape(128, 2048))
        in_maps.append({
            "xs": np.ascontiguousarray(xst[0:256]).astype(bf),
            "x8": np.ascontiguousarray(xst[256:512]).astype(f8),
            "wqs": wqs.astype(bf), "wks": wks.astype(bf), "wvs": wvs.astype(bf),
            "wq8s": wqs.astype(f8), "wk8s": wks.astype(f8),
            "wv8s": wvs.astype(f8),
        })
    return in_maps


def kernel(x, W_qkv, _res_hook=None):
    nc = _get_nc()
    in_maps = make_in_maps(x, W_qkv)
    res = run_bass_kernel_spmd(nc, in_maps, list(range(8)))
    if _res_hook is not None:
        _res_hook(res)
    out = np.empty((B, N, D), dtype=np.float32)
    for core in range(8):
        b, hg = core // 4, core % 4
        out[b, :, hg * 256:(hg + 1) * 256] = res.results[core]["outT"].T
    return out


# revision 84
# speedup vs baseline: 1.0853x; 1.0782x over previous
"""Causal self-attention Trainium2 Bass kernel.

Problem: B=2, N=2048, H=16 heads, Dh=64, D=1024, fp32.
  qkv = x @ W_qkv; causal softmax(q k^T / sqrt(Dh)) @ v.

Sharding (8 cores): data-parallel on B (2) x tensor-parallel on head groups (4).
Core c handles batch b = c // 4 and heads hg*4 .. hg*4+3 where hg = c % 4.

Host-side staging (make_in_maps) pre-arranges every input into the exact SBUF
image its tile wants, so each tensor loads with ONE multi-dim DMA descriptor
(the HW DGE fans a single descriptor across all 16 DMA engines; per-descriptor
issue is ~650ns on the sequencers, and the framework's DMA-semaphore pool is
only ~10 deep, so descriptor COUNT is precious):
  xs  [2*128, 4096]  f32 chunk-major x image (i-chunks 0-1 of 512 tokens)
  x8  [2*128, 4096]  fp8e4 image of chunks 2-3
  wqs/wks [128,2048] f32 pair-major; wvs [128,2048] f32 t-major
  wq8s/wk8s/wv8s     fp8e4 copies of the same images
  outT [256, 2048]   row h*64+d, col i = out[b, i, hg*256 + h*64 + d]
Input DMAs issue from BOTH HWDGE sequencers (sync + scalar) in strict
first-need order per FIFO; xt0 is split into i-quarters so the first QK
half-pieces (256 cols) can start ~4us earlier.

Precision plan (tolerance 2e-2; measured rel err ~9e-3): rows attending few
positions (chunk 0, and chunk 1 partially) are error-critical because softmax
noise doesn't average out there; long rows are robust.
  chunk 0:    f32r QKV, bf16 S, bf16 AV  (exact path)
  chunk 1:    f32r QKV, bf16 S, fp8e4 DoubleRow AV
  chunks 2-3: fp8e4 DoubleRow QKV (x8@W8), bf16 S, fp8e4 DoubleRow AV
DoubleRow packs 2 K-tiles per pass (2x rate) — it only pays on contraction-
deep matmuls (QKV K=1024, AV K=512), NOT on S (K=64/head, column-bound).
fp8 exp tops out at ~240-448, so the fp8-path EXP carries bias=-3 (softmax is
shift-invariant; numerator and denominator both scale by e^-3).

Device algorithm per core:
  qT/kT  [dh, i] tiles via matmul(lhsT=W-slice, rhs=xT), stored bf16
  v      [i, dh] tiles stored as v-hat = [v | ones64] (fp8, + bf16 copy of
         chunk 0): the ones rows replicate the softmax denominator into the
         AV output at zero PE cost (cycles only depend on N columns)
  S^T    [j, i] via row-packed bf16 matmul pairs (K=64/head, tile_position)
  expS^T via ACT Exp (scale=1/8, fp8-path bias=-3) straight from PSUM;
         chunks>=1 write fp8 pair-tiles [128, 2 jt, 2 l, 512] for DoubleRow
  causal mask on diagonal j-tiles via gpsimd affine_select (fill 0; full-width
         on fp8 tiles so never-written leading cols are zeroed for DR streams)
  AV     out^T accumulated in PSUM; fp8 path contracts 2 j-tiles per pass
  fin    per (l, col-group): DVE copy of the denominator rows, fast
         reciprocal, multiply, out-DMA (alternating sync/scalar engines)

Scheduling: a static greedy scheduler interleaves the S/EXP, QKV, and AV
streams against estimated PE/ACT/DMA clocks; emission order is the only knob
(real sync is inserted by the Tile framework).  Constraints encoded:
  - ps_s has 2 slots, so ACT runs at most ~2 EXPs ahead; feed it S units
    regularly and never >2 in a row
  - AV pairs chain per psum pool; fins are emitted eagerly (they cost no PE
    time and un-gate the pool chains)
  - front QKV pieces gate on FULL DMA arrival; dependency-free dummy matmuls
    fill predicted PE waits so the HAM clock gate never drops to 1.2 GHz
Engine budget (measured): PE matmul ~185us sum incl. ramp, ACT EXP ~80us,
DVE ~60us; HW exec ~139us vs the 157us all-f32r/bf16 baseline.
"""

import numpy as np

import concourse.mybir as mybir
import concourse.tile as tile
from concourse import bacc
from concourse.bass_utils import run_bass_kernel_spmd

F32 = mybir.dt.float32
F32R = mybir.dt.float32r
BF16 = mybir.dt.bfloat16
FP8 = mybir.dt.float8e4
DR = mybir.MatmulPerfMode.DoubleRow

B = 2
N = 2048
D = 1024
H_PER_CORE = 4
DH = 64
NCHUNK = 4          # i-chunks of 512
CH = 512
DT = 8              # d-tiles of 128
NT = 16             # token tiles of 128
SCALE = 1.0 / 8.0   # 1/sqrt(64)
E_BUFS = 40

_CACHED_NC = None


def build_nc(debug_sched=False, skip_compile=False):
    nc = bacc.Bacc("TRN2", target_bir_lowering=False, debug=False)
    # x chunks 0-1 + weights in f32; chunks 2-3 and a weight copy in fp8e4
    # (those rows average over 1024+ positions, so fp8 QKV noise washes out)
    xs = nc.dram_tensor("xs", [2 * 128, DT * CH], F32R, kind="ExternalInput").ap()
    x8 = nc.dram_tensor("x8", [2 * 128, DT * CH], FP8, kind="ExternalInput").ap()
    wqs = nc.dram_tensor("wqs", [128, 2 * DT * 128], F32R, kind="ExternalInput").ap()
    wks = nc.dram_tensor("wks", [128, 2 * DT * 128], F32R, kind="ExternalInput").ap()
    wvs = nc.dram_tensor("wvs", [128, DT * 256], F32R, kind="ExternalInput").ap()
    wq8s = nc.dram_tensor("wq8s", [128, 2 * DT * 128], FP8, kind="ExternalInput").ap()
    wk8s = nc.dram_tensor("wk8s", [128, 2 * DT * 128], FP8, kind="ExternalInput").ap()
    wv8s = nc.dram_tensor("wv8s", [128, DT * 256], FP8, kind="ExternalInput").ap()
    outT = nc.dram_tensor("outT", [H_PER_CORE * DH, N], F32, kind="ExternalOutput").ap()

    with tile.TileContext(nc) as tc:
        with (
            tc.tile_pool(name="sb_w", bufs=1) as sb_w,
            tc.tile_pool(name="sb_x", bufs=2) as sb_x,
            tc.tile_pool(name="sb_qk", bufs=1) as sb_qk,
            tc.tile_pool(name="sb_v", bufs=1) as sb_v,
            tc.tile_pool(name="sb_e", bufs=8) as sb_e,
            tc.tile_pool(name="sb_e8", bufs=30) as sb_e8,
            tc.tile_pool(name="sb_n", bufs=2) as sb_n,
            tc.tile_pool(name="ps_av", bufs=2, space="PSUM") as ps_av,
            tc.tile_pool(name="ps_qkv", bufs=2, space="PSUM") as ps_qkv,
            tc.tile_pool(name="ps_s", bufs=2, space="PSUM") as ps_s,
        ):
            wq_sb = sb_w.tile([128, 2 * DT * 128], F32R)
            wk_sb = sb_w.tile([128, 2 * DT * 128], F32R)
            wv_sb = sb_w.tile([128, DT * 256], F32R)
            wq8_sb = sb_w.tile([128, 2 * DT * 128], FP8)
            wk8_sb = sb_w.tile([128, 2 * DT * 128], FP8)
            wv8_sb = sb_w.tile([128, DT * 256], FP8)
            xtc8_tiles = {
                2: sb_w.tile([128, DT * CH], FP8, name="xtc8_2"),
                3: sb_w.tile([128, DT * CH], FP8, name="xtc8_3"),
            }

            xtc_tiles = {}

            # --- prologue DMAs: BOTH HWDGE groups (sync + scalar) are loaded
            # with the full input stream in strict first-need order; each
            # group's FIFO then delivers in priority order, so later tensors
            # can't steal bandwidth from the critical first pieces.  Tensors
            # are split between (or across) the two streams to balance them.
            xtc0 = sb_x.tile([128, DT * CH], F32R, tag="xtc", name="xtc0")
            xtc_tiles[0] = xtc0
            # xt0 split by i-halves (NOT by t): each half is a full-K image of
            # 256 i-columns, so the first QK half-pieces can start on half A.
            xtc0_v = xtc0[:, :].rearrange("p (t i) -> p t i", i=CH)
            xs0_v = xs[0:128, :].rearrange("p (t i) -> p t i", i=CH)
            # 12 descriptors total: the framework's DMA-semaphore pool is ~10
            # deep, so more would serialize issue on semaphore recycling.
            # First data for the q00a half-piece (xt0 i 0:256 + wq half 0)
            # leads both FIFOs.
            nc.sync.dma_start(xtc0_v[:, :, 0:128], xs0_v[:, :, 0:128])
            nc.scalar.dma_start(xtc0_v[:, :, 128:256], xs0_v[:, :, 128:256])
            nc.sync.dma_start(wq_sb[:, 0:1024], wqs[:, 0:1024])
            nc.scalar.dma_start(wk_sb[:, 0:1024], wks[:, 0:1024])
            nc.sync.dma_start(xtc0_v[:, :, 256:384], xs0_v[:, :, 256:384])
            nc.scalar.dma_start(xtc0_v[:, :, 384:CH], xs0_v[:, :, 384:CH])
            nc.sync.dma_start(wq_sb[:, 1024:2048], wqs[:, 1024:2048])
            nc.scalar.dma_start(wk_sb[:, 1024:2048], wks[:, 1024:2048])
            nc.scalar.dma_start(wv_sb[:, :], wvs[:, :])
            xtc1 = sb_x.tile([128, DT * CH], F32R, tag="xtc", name="xtc1")
            xtc_tiles[1] = xtc1
            nc.sync.dma_start(xtc1[:, :], xs[128:256, :])
            nc.scalar.dma_start(wq8_sb[:, :], wq8s[:, :])
            nc.scalar.dma_start(wk8_sb[:, :], wk8s[:, :])
            nc.sync.dma_start(xtc8_tiles[2][:, :], x8[0:128, :])
            nc.sync.dma_start(xtc8_tiles[3][:, :], x8[128:256, :])
            nc.scalar.dma_start(wv8_sb[:, :], wv8s[:, :])

            # warm-up: dependency-free matmuls on zeroed SBUF lift the HAM
            # clock gate (needs ~3.4us of PE activity); memset straight into
            # F32R tiles (gpsimd, earliest-starting engine) - no cast chain.
            wz = sb_v.tile([128, 1], F32)
            nc.gpsimd.memset(wz[:], 0.0)
            ebias_sb = sb_v.tile([128, 1], F32)
            nc.gpsimd.memset(ebias_sb[:], -3.0)
            xz = sb_v.tile([128, CH], F32)
            nc.gpsimd.memset(xz[:], 0.0)
            warm_ps = ps_qkv.tile([128, CH], F32, tag="ps_qkv")
            for _ in range(14):
                nc.tensor.matmul(warm_ps[0:1, :], wz[:].bitcast(F32R), xz[:].bitcast(F32R),
                                 start=True, stop=True, skip_group_check=True)

            # persistent activations (bf16 q/k: same PE rate as f32r, half the
            # SBUF; S is column-throughput-bound so fp8 wouldn't help it)
            qt_sb = sb_qk.tile([128, 2 * N], BF16)   # [pair][chunk]
            kt_sb = sb_qk.tile([128, 2 * N], BF16)
            # v-hat per (it, head): fp8 for chunks>=1 (DoubleRow AV), bf16
            # copy of chunk-0's tiles for the exact c0 path
            vh8_sb = sb_v.tile([128, NT * H_PER_CORE * 128], FP8)
            vh8_v = vh8_sb.rearrange("p (j h w) -> p j h w", h=H_PER_CORE, w=128)
            vh16_sb = sb_v.tile([128, 4 * H_PER_CORE * 128], BF16)
            # ones columns of v-hat (denominator trick): one strided memset
            nc.vector.memset(vh8_v[:, :, :, 64:128], 1.0)
            vh16_blocks = vh16_sb.rearrange("p (b w) -> p b w", w=128)
            nc.vector.memset(vh16_blocks[:, :, 64:128], 1.0)

            # ---------------- emission thunks ----------------

            def qk_piece(c, p, which, lo=0, hi=CH):
                w = hi - lo
                dst = qt_sb if which == "q" else kt_sb
                pres = ps_qkv.tile([128, CH], F32, tag="ps_qkv")
                if c >= 2:
                    # fp8 DoubleRow: t-pairs contract K=256 per pass (2x rate)
                    xt8 = xtc8_tiles[c].rearrange("r (t i) -> r t i", i=CH)
                    w8 = wq8_sb if which == "q" else wk8_sb
                    for tp in range(4):
                        lhsT = w8[:, p * 1024 + tp * 256: p * 1024 + (tp + 1) * 256
                                  ].rearrange("r (t c2) -> r t c2", t=2)
                        nc.tensor.matmul(
                            pres[:, 0:w], lhsT, xt8[:, 2 * tp:2 * tp + 2, lo:hi],
                            start=(tp == 0), stop=(tp == 3), perf_mode=DR)
                else:
                    xtc = xtc_tiles[c]
                    w_sb = wq_sb if which == "q" else wk_sb
                    for t in range(DT):
                        nc.tensor.matmul(
                            pres[:, 0:w], w_sb[:, p * 1024 + t * 128: p * 1024 + (t + 1) * 128],
                            xtc[:, t * CH + lo: t * CH + hi],
                            start=(t == 0), stop=(t == DT - 1))
                nc.vector.tensor_copy(
                    dst[:, p * N + c * CH + lo: p * N + c * CH + hi], pres[:, 0:w])

            def v_piece(c, il):
                it = 4 * c + il
                v_ps = ps_qkv.tile([128, 256], F32, tag="ps_qkv")
                if c >= 2:
                    xt8 = xtc8_tiles[c].rearrange("r (t i) -> r t i", i=CH)
                    for tp in range(4):
                        rhs = wv8_sb[:, tp * 512:(tp + 1) * 512
                                     ].rearrange("r (t c2) -> r t c2", t=2)
                        nc.tensor.matmul(
                            v_ps[:], xt8[:, 2 * tp:2 * tp + 2, il * 128:(il + 1) * 128],
                            rhs,
                            start=(tp == 0), stop=(tp == 3), perf_mode=DR)
                else:
                    xtc = xtc_tiles[c]
                    for t in range(DT):
                        nc.tensor.matmul(
                            v_ps[:], xtc[:, t * CH + il * 128: t * CH + (il + 1) * 128],
                            wv_sb[:, t * 256:(t + 1) * 256],
                            start=(t == 0), stop=(t == DT - 1))
                # one strided cast: 4 heads' v blocks (cols 0:64 of each 128-block)
                nc.vector.tensor_copy(
                    vh8_v[:, it, :, 0:64],
                    v_ps.rearrange("p (h w) -> p h w", h=4)[:])
                if c == 0:   # exact bf16 copy for the chunk-0 AV path
                    dst = vh16_sb[:, it * 512:(it + 1) * 512]
                    nc.vector.tensor_copy(
                        dst.rearrange("p (h w) -> p h w", h=4)[:, :, 0:64],
                        v_ps.rearrange("p (h w) -> p h w", h=4)[:])

            e_tiles = {}     # c==0: bf16 [128, 1024] per (c,p,jt)
            e8_pairs = {}    # c>=1: fp8 [128, 2048] per (c,p,jt//2), layout [jtslot][l][i]

            def s_exp_jt(c, p, jt):
                d = jt - 4 * c           # >= 0 on diagonal tiles
                off = 128 * d if d > 0 else 0
                s_ps = ps_s.tile([128, 1024], F32, tag="ps_s",
                                 name=f"s_c{c}_p{p}_j{jt}")
                for l in range(2):
                    nc.tensor.matmul(
                        s_ps[:, l * CH + off:(l + 1) * CH],
                        kt_sb[l * 64:(l + 1) * 64, p * N + jt * 128: p * N + (jt + 1) * 128],
                        qt_sb[l * 64:(l + 1) * 64, p * N + c * CH + off: p * N + (c + 1) * CH],
                        start=True, stop=True,
                        tile_position=(l * 64, 0))
                if c == 0:
                    e_t = sb_e.tile([128, 1024], BF16, tag="e",
                                    name=f"e_c{c}_p{p}_j{jt}")
                    e_view = e_t[:, :]
                    e_tiles[(c, p, jt)] = e_t
                else:
                    if jt % 2 == 0:
                        e8_pairs[(c, p, jt // 2)] = sb_e8.tile(
                            [128, 2048], FP8, tag="e8", name=f"e8_c{c}_p{p}_j{jt}")
                    e_view = e8_pairs[(c, p, jt // 2)][:, (jt % 2) * 1024:(jt % 2) * 1024 + 1024]
                # fp8 e4m3 tops out at 448 < e^{max S}: bias the fp8-path exp
                # by -3 (softmax is shift-invariant; num and den both carry
                # e^-3, cancelling in the ratio)
                ebias = 0.0 if c == 0 else ebias_sb[:, 0:1]
                if off:
                    # one ACT instruction over both l-halves, skipping the
                    # fully-masked leading columns of each half
                    src = s_ps.rearrange("p (l w) -> p l w", l=2)[:, :, off:]
                    dst = e_view.rearrange("p (l w) -> p l w", l=2)[:, :, off:]
                    nc.scalar.activation(dst, src,
                                         mybir.ActivationFunctionType.Exp,
                                         scale=SCALE, bias=ebias)
                else:
                    nc.scalar.activation(e_view, s_ps[:],
                                         mybir.ActivationFunctionType.Exp,
                                         scale=SCALE, bias=ebias)
                if d >= 0:  # diagonal tile: zero where j > i
                    # c>=1 (fp8 DR) streams the FULL width, so the leading
                    # [0:off) cols must be zero-filled too: run the select
                    # full-width with base=-off (it reads-then-discards the
                    # never-written cols, writing fill there).
                    wsel = CH if c != 0 else CH - off
                    o2 = 0 if c != 0 else off
                    for l in range(2):
                        nc.gpsimd.affine_select(
                            out=e_view[:, l * CH + o2:(l + 1) * CH],
                            in_=e_view[:, l * CH + o2:(l + 1) * CH],
                            compare_op=mybir.AluOpType.is_ge,
                            fill=0.0,
                            base=-off + o2,
                            channel_multiplier=-1,
                            pattern=[[1, wsel]])

            av_tiles = {}

            def setup_pair(c, p):
                # late pairs borrow by-then-idle psum pools so multiple pairs
                # accumulate concurrently at the endgame; (3,1) gets the endgame
                pool, tag = ((ps_qkv, "ps_qkv") if (c, p) in QKV_POOL_PAIRS
                             else (ps_av, "ps_av"))
                for l in range(2):
                    av_tiles[(c, p, l)] = pool.tile(
                        [128, CH], F32, tag=tag, name=f"av_c{c}_p{p}_l{l}")

            def av_half(c, p, jt0, l):
                njt = 4 * (c + 1)
                h = p * 2 + l
                if c == 0:   # exact bf16 path
                    for jt in range(jt0, jt0 + 4):
                        dd = jt - 4 * c
                        off = 128 * dd if dd > 0 else 0
                        e_t = e_tiles[(c, p, jt)]
                        nc.tensor.matmul(
                            av_tiles[(c, p, l)][:, off:],
                            vh16_sb[:, (jt * H_PER_CORE + h) * 128: (jt * H_PER_CORE + h + 1) * 128],
                            e_t[:, l * CH + off:(l + 1) * CH],
                            start=(jt == 0),
                            stop=(jt == njt - 1),
                            skip_group_check=True)
                    return
                # fp8 DoubleRow: one matmul per jt PAIR (K = 2x128), full
                # width (masked e cols are zero-filled)
                for jp in (jt0 // 2, jt0 // 2 + 1):
                    e8 = e8_pairs[(c, p, jp)]
                    rhs = e8.rearrange("p (j l i) -> p j l i", j=2, l=2)[:, :, l, :]
                    nc.tensor.matmul(
                        av_tiles[(c, p, l)][:, :],
                        vh8_v[:, 2 * jp:2 * jp + 2, h, :],
                        rhs,
                        start=(jp == 0),
                        stop=(jp == njt // 2 - 1),
                        perf_mode=DR,
                        skip_group_check=True)

            fin_left = {}

            def fin_part(c, p, l, g, w, engine):
                # NOTE: the partition shift (rows 64:128 -> 0:64) must go through
                # tensor_copy; custom-DVE ops don't partition-shift on real HW.
                h = p * 2 + l
                av_t = av_tiles[(c, p, l)]
                s0 = g * w
                sl = slice(s0, s0 + w)
                sums_sb = sb_n.tile([64, CH], F32, tag="sums")
                nc.vector.tensor_copy(sums_sb[:, 0:w], av_t[64:128, sl])
                rc = sb_n.tile([64, CH], F32, tag="rc")
                nc.vector.reciprocal_approx_fast(rc[:, 0:w], sums_sb[:, 0:w])
                out_sb = sb_n.tile([64, CH], F32, tag="out")
                nc.vector.tensor_mul(out_sb[:, 0:w], av_t[0:64, sl], rc[:, 0:w])
                engine.dma_start(
                    outT[h * 64:(h + 1) * 64, c * CH + s0: c * CH + s0 + w],
                    out_sb[:, 0:w])
                fin_left[(c, p)] -= 1
                if fin_left[(c, p)] == 0:
                    for l2 in range(2):
                        av_tiles.pop((c, p, l2))

            # ---------------- static greedy scheduler ----------------
            # Emission order is the only knob: real sync is inserted by the
            # Tile framework.  Estimated clocks in us.
            S_COST = {0: 0.48, 1: 0.38, 2: 0.28, 3: 0.18}
            QK_COST = 1.75
            V_COST = 0.9
            AV_COST = 0.88
            EXP_COST = {0: 1.15, 1: 0.94, 2: 0.72, 3: 0.51}
            AS_LAT = 0.62
            ACT_MIN_BACKLOG = 2.4   # keep >= ~2 EXPs queued on ACT
            AV_MARGIN = 0.9

            # DMA-completion-based readiness estimates (single-descriptor DMAs,
            # ~0.30 GB/us aggregate from ~7.6us).  wv and xtc2 are kicked from
            # inside the EXP stream (see emit_s); xtc3 from sync after chunk 0
            # drains.
            qkv_queue = [
                ("q", (0, 0, 0, 256), 16.4), ("k", (0, 0, 0, 256), 16.8),
                ("q", (0, 0, 256, 512), 20.6), ("k", (0, 0, 256, 512), 21.0),
                ("q", (0, 1, 0, 256), 21.4), ("k", (0, 1, 0, 256), 21.8),
                ("q", (0, 1, 256, 512), 22.2), ("k", (0, 1, 256, 512), 22.6),
                ("v", (0, 0), 28.5), ("v", (0, 1), 28.7),
                ("v", (0, 2), 28.9), ("v", (0, 3), 29.1),
                ("q", (1, 0), 29.5), ("k", (1, 0), 30.0),
                ("q", (1, 1), 30.5), ("k", (1, 1), 31.0),
                ("v", (1, 0), 31.4), ("v", (1, 1), 31.6),
                ("q", (2, 0), 34.0), ("k", (2, 0), 34.3),
                ("q", (2, 1), 34.6), ("k", (2, 1), 34.9),
                ("v", (1, 2), 35.2), ("v", (1, 3), 35.4),
                ("v", (2, 0), 35.7), ("v", (2, 1), 35.9),
                ("q", (3, 0), 36.6), ("k", (3, 0), 36.9),
                ("v", (2, 2), 37.1), ("v", (2, 3), 37.3),
                ("q", (3, 1), 37.6), ("k", (3, 1), 37.9),
                ("v", (3, 0), 38.2), ("v", (3, 1), 38.4),
                ("v", (3, 2), 38.6), ("v", (3, 3), 38.8),
            ]

            s_queue = []
            for c in range(NCHUNK):
                for p in range(2):
                    for jt in range(4 * (c + 1)):
                        s_queue.append((c, p, jt))

            # Per-pair AV queues.  Pairs on different psum pools can interleave;
            # same-pool pairs are chained (setup allocates the pool slots, so it
            # must follow the predecessor pair's last fin).  ps_qkv-hosted pairs
            # also wait for the last QKV allocation.
            PAIR_ORDER = [(0, 0), (0, 1), (1, 0), (1, 1),
                          (2, 0), (2, 1), (3, 0), (3, 1)]
            QKV_POOL_PAIRS = ((2, 1), (3, 1))
            pair_pred = {}
            last_on = {"ps_av": None, "ps_qkv": None}
            for pr in PAIR_ORDER:
                pool = "ps_qkv" if pr in QKV_POOL_PAIRS else "ps_av"
                pair_pred[pr] = last_on[pool]
                last_on[pool] = pr
            pair_items = {}
            for (c, p) in PAIR_ORDER:
                items = [("setup", None)]
                njt = 4 * (c + 1)
                nfin = 2 if c >= 2 else 1   # fin reads split 256-wide on late chunks
                for jt0 in range(0, njt, 4):
                    last = jt0 == njt - 4
                    for l in range(2):
                        items.append(("av", (jt0, l)))
                        if last:
                            for g in range(nfin):
                                items.append(("fin", (l, g, CH // nfin)))
                pair_items[(c, p)] = items
                fin_left[(c, p)] = 2 * nfin
            pair_idx = {pr: 0 for pr in PAIR_ORDER}
            fins_emitted = set()

            qk_done = {}        # (c, p, 'q'/'k') -> est emission T_pe
            v_done = set()      # (c, il) emitted
            act_done = {}       # s unit -> est ACT completion time
            T_pe = 13.8         # est PE clock after warmup
            T_act = 0.0         # ACT busy-until
            e_inflight = 0
            qi = si = 0
            sched_log = []

            def emit_dummies(gap):
                """Fill an estimated PE stall with dependency-free matmuls so
                the HAM clock gate never sees an idle window.  Cold dummies
                (front, pre-HAM-lift) run ~0.43us, warm ~0.21us."""
                nonlocal T_pe
                cost = 0.43 if T_pe < 20.0 else 0.21
                n = min(26, int(gap * 1.02 / cost))
                if n <= 0:
                    return
                dtile = ps_s.tile([128, 1024], F32, tag="ps_s", name="dummy")
                for _ in range(n):
                    nc.tensor.matmul(dtile[0:1, 0:512],
                                     wz[:].bitcast(F32R), xz[:].bitcast(F32R),
                                     start=True, stop=True, skip_group_check=True)
                T_pe += n * cost
                sched_log.append((round(T_pe, 1), "dummy", n))

            def s_ready():
                c, p, jt = s_queue[si]
                if e_inflight >= E_BUFS - 2:
                    return False
                if (c, p, "q") not in qk_done or (jt // 4, p, "k") not in qk_done:
                    return False
                # PSUM->SBUF copy of qt/kt trails the piece matmuls by ~0.9us
                return T_pe >= max(qk_done[(c, p, "q")], qk_done[(jt // 4, p, "k")]) + 0.9

            def av_total_left():
                return sum(len(pair_items[pr]) - pair_idx[pr] for pr in PAIR_ORDER)

            def pair_open(pr):
                if pair_pred[pr] is not None and pair_pred[pr] not in fins_emitted:
                    return False
                if pr in QKV_POOL_PAIRS and qi < len(qkv_queue):
                    return False
                return True

            def next_av(relaxed=False):
                """Pending fins first (they cost no PE time and un-gate the
                psum chains), then the ready pair with the MOST progress —
                racing pairs to completion gets their fins (DVE normalize +
                out-DMA) flowing while later pairs still have PE work."""
                for pr in PAIR_ORDER:
                    idx = pair_idx[pr]
                    if idx < len(pair_items[pr]) and pair_items[pr][idx][0] == "fin":
                        return pr
                for pr in sorted(PAIR_ORDER, key=lambda pr: (pair_idx[pr], PAIR_ORDER.index(pr))):
                    idx = pair_idx[pr]
                    items = pair_items[pr]
                    if idx >= len(items):
                        continue
                    kind, args = items[idx]
                    if kind == "setup":
                        if not pair_open(pr):
                            continue
                        # setup is free; look through to the first av half
                        if len(items) > 1:
                            kind, args = items[idx + 1]
                        else:
                            return pr
                    if kind == "fin":
                        return pr
                    jt0, l = args
                    c, p = pr
                    if any((c, p, jt) not in act_done for jt in range(jt0, jt0 + 4)):
                        continue
                    if any((jt0 // 4, il) not in v_done for il in range(4)):
                        continue
                    if not relaxed and not (
                            act_done[(c, p, jt0 + 3)] + AS_LAT <= T_pe + AV_MARGIN):
                        continue
                    return pr
                return None

            def emit_qkv():
                nonlocal qi, T_pe
                kind, args, rt = qkv_queue[qi]
                qi += 1
                if rt > T_pe:
                    emit_dummies(rt - T_pe)
                T_pe = max(T_pe, rt)
                if kind in ("q", "k"):
                    if len(args) == 4:
                        c, p, lo, hi = args
                        qk_piece(c, p, kind, lo, hi)
                        T_pe += QK_COST * (hi - lo) / CH
                        if hi == CH:
                            qk_done[(c, p, kind)] = T_pe
                    else:
                        c, p = args
                        qk_piece(c, p, kind)
                        T_pe += QK_COST if c < 2 else QK_COST * 0.55
                        qk_done[(c, p, kind)] = T_pe
                else:
                    c, il = args
                    v_piece(c, il)
                    v_done.add((c, il))
                    T_pe += V_COST if c < 2 else V_COST * 0.55
                sched_log.append((round(T_pe, 1), kind, args))

            act_done_seq = []   # est ACT completion per emitted s unit, in order
            act_idle = [0.0]

            def emit_s():
                nonlocal si, T_pe, T_act, e_inflight
                unit = s_queue[si]
                si += 1
                c, p, jt = unit
                s_exp_jt(c, p, jt)
                d = jt - 4 * c
                # ps_s has 2 slots: this unit's matmuls wait until the EXP two
                # units ago has drained its slot
                if len(act_done_seq) >= 2:
                    T_pe = max(T_pe, act_done_seq[-2])
                T_pe += S_COST.get(d, 0.48)
                if T_act > 0 and T_pe > T_act:
                    act_idle[0] += T_pe - T_act
                T_act = max(T_act, T_pe) + EXP_COST.get(d, 1.15)
                act_done[unit] = T_act
                act_done_seq.append(T_act)
                e_inflight += 1
                sched_log.append((round(T_pe, 1), "s", unit, round(T_act, 1)))

            def emit_av(pr, relaxed=False):
                nonlocal T_pe, e_inflight
                c, p = pr
                kind, args = pair_items[pr][pair_idx[pr]]
                pair_idx[pr] += 1
                if kind == "setup":
                    setup_pair(c, p)
                    if pair_idx[pr] < len(pair_items[pr]):
                        emit_av(pr, relaxed)   # fall through to the first half
                elif kind == "fin":
                    l, g, w = args
                    # late fins alternate out-DMA engines: the sync HWDGE
                    # group processes descriptors serially, so the 2MB output
                    # stream would otherwise drain ~6us past the last matmul
                    eng = nc.scalar if (c >= 2 and g == 1) else nc.sync
                    fin_part(c, p, l, g, w, eng)
                    sched_log.append((round(T_pe, 1), "fin", (c, p, l, g)))
                else:
                    jt0, l = args
                    if relaxed:
                        t_need = act_done[(c, p, jt0 + 3)] + AS_LAT
                        if t_need > T_pe:
                            emit_dummies(t_need - T_pe)
                        T_pe = max(T_pe, t_need)
                    av_half(c, p, jt0, l)
                    if l == 1:
                        e_inflight -= 4
                    T_pe += AV_COST if c == 0 else AV_COST * 0.52
                    sched_log.append((round(T_pe, 1), "av", (c, p, jt0, l)))
                if pair_idx[pr] >= len(pair_items[pr]):
                    fins_emitted.add(pr)

            last_s_T = 0.0
            consec_s = 0
            other_toggle = 0
            while qi < len(qkv_queue) or si < len(s_queue) or av_total_left():
                # dma kicks are free: drain them whenever they reach the head
                while qi < len(qkv_queue) and qkv_queue[qi][0] in ("dma", "dmav"):
                    emit_qkv()
                backlog = T_act - T_pe
                s_ok = si < len(s_queue) and s_ready()
                qkv_ok = qi < len(qkv_queue) and T_pe >= qkv_queue[qi][2] - 0.5
                av_pr = next_av()
                # ACT holds at most ~2 runnable EXPs (2 ps_s slots), so it must
                # be re-fed at least every ~1.8us of PE work; and >2 s units in
                # a row stall the PE on EXP drain.
                s_want = backlog < ACT_MIN_BACKLOG or T_pe - last_s_T > 1.8
                if s_ok and s_want and (consec_s < 2 or not (qkv_ok or av_pr)):
                    emit_s()
                    last_s_T = T_pe
                    consec_s += 1
                    continue
                e_pressure = e_inflight >= E_BUFS - 8
                if e_pressure and av_pr is not None:
                    emit_av(av_pr)
                elif qkv_ok and av_pr is not None:
                    # both streams ready: alternate so neither the QKV stream
                    # nor the 6-pair-deep ps_av chain falls behind
                    if other_toggle % 2 == 0:
                        emit_qkv()
                    else:
                        emit_av(av_pr)
                    other_toggle += 1
                elif qkv_ok:
                    emit_qkv()
                elif av_pr is not None:
                    emit_av(av_pr)
                elif s_ok:
                    emit_s()
                    last_s_T = T_pe
                elif qi < len(qkv_queue):
                    emit_qkv()   # forced: waits on DMA est
                else:
                    pr = next_av(relaxed=True)
                    if pr is None:
                        raise RuntimeError("scheduler stuck")
                    emit_av(pr, relaxed=True)
                consec_s = 0

            if debug_sched:
                for entry in sched_log:
                    print(entry)
                print(f"est T_pe={T_pe:.1f} T_act={T_act:.1f} act_idle={act_idle[0]:.1f}")

    if not skip_compile:
        nc.compile()
    return nc


def _get_nc():
    global _CACHED_NC
    if _CACHED_NC is None:
        _CACHED_NC = build_nc()
    return _CACHED_NC


def make_in_maps(x, W_qkv):
    import ml_dtypes
    f8 = ml_dtypes.float8_e4m3
    x = np.ascontiguousarray(np.asarray(x, dtype=np.float32))
    W = np.ascontiguousarray(np.asarray(W_qkv, dtype=np.float32))
    in_maps = []
    for core in range(8):
        b, hg = core // 4, core % 4
        cols = slice(hg * 256, (hg + 1) * 256)
        xT = x[b].T                                    # [1024, 2048]
        xst = xT.reshape(8, 128, 4, 512).transpose(2, 1, 0, 3).reshape(512, 4096)
        Wq = W[:, 0 * D:1 * D][:, cols]
        Wk = W[:, 1 * D:2 * D][:, cols]
        Wv = W[:, 2 * D:3 * D][:, cols]
        wqs = np.ascontiguousarray(
            Wq.reshape(8, 128, 2, 128).transpose(1, 2, 0, 3).reshape(128, 2048))
        wks = np.ascontiguousarray(
            Wk.reshape(8, 128, 2, 128).transpose(1, 2, 0, 3).reshape(128, 2048))
        wvs = np.ascontiguousarray(
            Wv.reshape(8, 128, 256).transpose(1, 0, 2).reshape(128, 2048))
        in_maps.append({
            "xs": np.ascontiguousarray(xst[0:256]),
            "x8": np.ascontiguousarray(xst[256:512]).astype(f8),
            "wqs": wqs, "wks": wks, "wvs": wvs,
            "wq8s": wqs.astype(f8), "wk8s": wks.astype(f8),
            "wv8s": wvs.astype(f8),
        })
    return in_maps


def kernel(x, W_qkv, _res_hook=None):
    nc = _get_nc()
    in_maps = make_in_maps(x, W_qkv)
    res = run_bass_kernel_spmd(nc, in_maps, list(range(8)))
    if _res_hook is not None:
        _res_hook(res)
    out = np.empty((B, N, D), dtype=np.float32)
    for core in range(8):
        b, hg = core // 4, core % 4
        out[b, :, hg * 256:(hg + 1) * 256] = res.results[core]["outT"].T
    return out
